# revision 39
# baseline (speedup 1.0000x reference)
"""GroupMamba block kernel for TRN2 — per-core body + host weight prep.

Per-core work: 2 batches of the (16, 3136, 256) problem. Layout is
channel-partition: activations live as [128 ch, L=3136] tiles, one per
(batch, ctile). All cross-partition ops (LN stats, broadcasts, dwconv,
projections) go through the TensorEngine with host-built block matrices.
The Mamba recurrence is a single tensor_tensor_scan per group.
"""
import numpy as np
from contextlib import ExitStack

import concourse.bass as bass
import concourse.tile as tile  # noqa: F401
from concourse import mybir

F32 = mybir.dt.float32
F32R = mybir.dt.float32r
BF16 = mybir.dt.bfloat16
AF = mybir.ActivationFunctionType
OP = mybir.AluOpType
AX = mybir.AxisListType

B = 2          # batches per core
T = 2          # ctiles (256 = 2*128)
G = 4          # ss2d groups
Cg = 64
C = 256
H = W = 56
L = H * W      # 3136
HID = 1024
HS = 8         # hidden slices of 128
CK = 448       # L-chunk (8 pixel rows)
NCK = L // CK  # 7
PW = 64        # padded row stride
PR = 58        # padded rows
LP = PR * PW   # 3712
EPS = 1e-5

VB_NAMES = ([f"A{g}" for g in range(G)] + [f"dtb{g}" for g in range(G)]
            + [f"cvb{g}" for g in range(G)] + [f"Dp{g}" for g in range(G)]
            + [f"onb{g}" for g in range(G)] + [f"n1b{t}" for t in range(T)]
            + [f"pjb{t}" for t in range(T)] + [f"f1b{s}" for s in range(HS)]
            + [f"f2b{t}" for t in range(T)] + [f"dwb{s}" for s in range(HS)]
            + [f"fcb{t}" for t in range(T)])
VB_IDX = {n: i for i, n in enumerate(VB_NAMES)}


def f32r(ap):
    return ap.bitcast(F32R)


# ---------------------------------------------------------------- host prep
def host_prep(x2b, w):
    """x2b: (2, 3136, 256) f32 shard; w: dict of full weights.
    Returns the per-core device input map (numpy arrays)."""
    import ml_dtypes
    bf = ml_dtypes.bfloat16
    N = np.float32

    def bfar(a):
        return np.ascontiguousarray(np.asarray(a, dtype=np.float32)).astype(bf)

    out = {}
    xt = np.asarray(x2b, dtype=N).transpose(0, 2, 1).reshape(B, T, 128, L)
    out["xt"] = np.ascontiguousarray(xt)

    n1w = np.asarray(w["norm1_w"], N); n1b = np.asarray(w["norm1_b"], N)
    n2w = np.asarray(w["norm2_w"], N); n2b = np.asarray(w["norm2_b"], N)
    skip = float(np.asarray(w["skip_scale"]).reshape(-1)[0])

    stF = np.zeros((128, 4), N)
    stF[:, 0] = 1.0 / C
    stF[:, 3] = 1.0 / C
    out["w_stF_f"] = stF
    out["w_stF_h"] = bfar(stF)
    stG = np.zeros((128, 2), N)
    stG[:64, 0] = 1.0 / Cg
    stG[64:, 1] = 1.0 / Cg
    out["w_stG"] = bfar(stG)

    bc1w = np.zeros((2, B * T * 128), N)
    for b in range(B):
        for t in range(T):
            bc1w[b, (b * T + t) * 128:(b * T + t + 1) * 128] = \
                n1w[t * 128:(t + 1) * 128]
    out["w_bc1w"] = bfar(bc1w)
    bci = np.zeros((2, B * 128), N)
    bci[0, :128] = 1.0
    bci[1, 128:] = 1.0
    out["w_bci"] = bfar(bci)
    bon = np.zeros((2, G * 128), N)
    onw = np.asarray(w["out_norm_w"], N)
    for g in range(G):
        bon[0, g * 128:g * 128 + 64] = onw[g]
        bon[1, g * 128 + 64:(g + 1) * 128] = onw[g]
    out["w_on"] = bfar(bon)

    ipw = np.asarray(w["in_proj_w"], N)
    ipx = np.zeros((128, G, 128), N)
    ipz = np.zeros((128, G, 128), N)
    for g in range(G):
        R = (g % 2) * 64
        bx = ipw[g][:64].T
        bz = ipw[g][64:].T
        ipx[R:R + 64, g, 0:64] = bx
        ipx[R:R + 64, g, 64:128] = bx
        ipz[R:R + 64, g, 0:64] = bz
        ipz[R:R + 64, g, 64:128] = bz
    out["w_ipx"] = bfar(ipx.reshape(128, G * 128))
    out["w_ipz"] = bfar(ipz.reshape(128, G * 128))

    cw = np.asarray(w["conv_w"], N)
    cvd = np.zeros((128, G, 9, 128), N)
    for g in range(G):
        for k in range(9):
            v = cw[g, :, k // 3, k % 3]
            cvd[np.arange(128), g, k, np.arange(128)] = np.concatenate([v, v])
    out["w_cv"] = bfar(cvd.reshape(128, G * 9 * 128))

    dww = np.asarray(w["dw_w"], N)
    cvh = np.zeros((128, HS, 9, 128), N)
    for s in range(HS):
        for k in range(9):
            v = dww[s * 128:(s + 1) * 128, k // 3, k % 3]
            cvh[np.arange(128), s, k, np.arange(128)] = v
    out["w_cvh"] = bfar(cvh.reshape(128, HS * 9 * 128))

    # x_proj and dt_proj folded into single per-group [64,64] / rank-1
    # matrices applied directly to u (no XD intermediate on device)
    xpw = np.asarray(w["x_proj_w"], N)
    dtw = np.asarray(w["dt_proj_w"], N)
    dtx = np.zeros((128, G, 128), N)
    bbl = np.zeros((128, G, 128), N)
    ccl = np.zeros((128, G, 128), N)
    for g in range(G):
        blk = (dtw[g] @ xpw[g][:4]).T          # [k, c]
        for b in range(2):
            sl = slice(b * 64, (b + 1) * 64)
            dtx[sl, g, sl] = blk
            bbl[sl, g, sl] = xpw[g][4][:, None]
            ccl[sl, g, sl] = xpw[g][5][:, None]
    out["w_dtx"] = bfar(dtx.reshape(128, G * 128))
    out["w_bb"] = bfar(bbl.reshape(128, G * 128))
    out["w_cc"] = bfar(ccl.reshape(128, G * 128))

    opw = np.asarray(w["out_proj_w"], N)
    opl = np.zeros((128, G, 64), N)
    for g in range(G):
        # extra 0.5: gate is computed as 1+tanh(v/2) = 2*sigmoid(v)
        blk = (opw[g] * skip * 0.5).T
        opl[0:64, g] = blk
        opl[64:128, g] = blk
    out["w_op"] = bfar(opl.reshape(128, G * 64))

    pw = np.asarray(w["proj_w"], N) * n1w[None, :]
    pj = np.zeros((128, T, T, 128), N)
    for t in range(T):
        for kt in range(T):
            pj[:, t, kt, :] = pw[t * 128:(t + 1) * 128,
                                 kt * 128:(kt + 1) * 128].T
    out["w_pj"] = bfar(pj.reshape(128, T * T * 128))
    pjb = np.asarray(w["proj_b"], N) + np.asarray(w["proj_w"], N) @ n1b

    f1w = np.asarray(w["fc1_w"], N) * n2w[None, :]
    f1 = np.zeros((128, T, HS, 128), N)
    for kt in range(T):
        for hs in range(HS):
            f1[:, kt, hs, :] = f1w[hs * 128:(hs + 1) * 128,
                                   kt * 128:(kt + 1) * 128].T
    out["w_f1"] = bfar(f1.reshape(128, T * HS * 128))
    f1b = np.asarray(w["fc1_b"], N) + np.asarray(w["fc1_w"], N) @ n2b

    f2w = np.asarray(w["fc2_w"], N)
    f2 = np.zeros((128, HS, T, 128), N)
    for hs in range(HS):
        for t in range(T):
            f2[:, hs, t, :] = f2w[t * 128:(t + 1) * 128,
                                  hs * 128:(hs + 1) * 128].T
    out["w_f2"] = bfar(f2.reshape(128, HS * T * 128))

    S = np.zeros((C, C), N)
    ca = np.asarray(w["ca_w"], N)
    for i in range(C):
        for d in range(3):
            j = i + d - 1
            if 0 <= j < C:
                S[i, j] += ca[d]
    fcs = (np.asarray(w["fc_w"], N) + S) / float(L)
    fl = np.zeros((128, T, T, 128), N)
    for kt in range(T):
        for t in range(T):
            fl[:, kt, t, :] = fcs[t * 128:(t + 1) * 128,
                                  kt * 128:(kt + 1) * 128].T
    out["w_fcs"] = bfar(fl.reshape(128, T * T * 128))

    cols = {}
    for g in range(G):
        cols[f"A{g}"] = -np.exp(np.asarray(w["A_log"], N)[g][:, 0])
        cols[f"dtb{g}"] = np.asarray(w["dt_proj_b"], N)[g]
        cols[f"cvb{g}"] = np.asarray(w["conv_b"], N)[g]
        cols[f"Dp{g}"] = np.asarray(w["Dp"], N)[g]
        cols[f"onb{g}"] = np.asarray(w["out_norm_b"], N)[g]
    for t in range(T):
        cols[f"n1b{t}"] = n1b[t * 128:(t + 1) * 128]
        cols[f"pjb{t}"] = pjb[t * 128:(t + 1) * 128]
        cols[f"f2b{t}"] = np.asarray(w["fc2_b"], N)[t * 128:(t + 1) * 128]
        # halved: gate uses tanh(v/2) so the bias enters pre-scaled by 0.5
        cols[f"fcb{t}"] = 0.5 * np.asarray(w["fc_b"], N)[t * 128:(t + 1) * 128]
    for s in range(HS):
        cols[f"f1b{s}"] = f1b[s * 128:(s + 1) * 128]
        cols[f"dwb{s}"] = np.asarray(w["dw_b"], N)[s * 128:(s + 1) * 128]
    vbm = np.zeros((128, len(VB_NAMES)), N)
    for n, i in VB_IDX.items():
        c = cols[n]
        vbm[:, i] = np.concatenate([c, c]) if c.shape[0] == 64 else c
    out["vb"] = vbm
    return out


def input_specs():
    """shapes/dtypes of the device inputs (excluding xt)."""
    import ml_dtypes
    bf = ml_dtypes.bfloat16
    N = np.float32
    return {
        "xt": ((B, T, 128, L), N),
        "w_stF_f": ((128, 4), N),
        "w_stF_h": ((128, 4), bf),
        "w_stG": ((128, 2), bf),
        "w_bc1w": ((2, B * T * 128), bf),
        "w_bci": ((2, B * 128), bf),
        "w_on": ((2, G * 128), bf),
        "w_ipx": ((128, G * 128), bf),
        "w_ipz": ((128, G * 128), bf),
        "w_cv": ((128, G * 9 * 128), bf),
        "w_cvh": ((128, HS * 9 * 128), bf),
        "w_dtx": ((128, G * 128), bf),
        "w_bb": ((128, G * 128), bf),
        "w_cc": ((128, G * 128), bf),
        "w_op": ((128, G * 64), bf),
        "w_pj": ((128, T * T * 128), bf),
        "w_f1": ((128, T * HS * 128), bf),
        "w_f2": ((128, HS * T * 128), bf),
        "w_fcs": ((128, T * T * 128), bf),
        "vb": ((128, len(VB_NAMES)), N),
    }


# ------------------------------------------------------------- device body
def body(ctx: ExitStack, tc, outs, ins):
    nc = tc.nc
    wb = ctx.enter_context(tc.tile_pool(name="wb", bufs=1))
    big = ctx.enter_context(tc.tile_pool(name="big", bufs=1))
    grp = ctx.enter_context(tc.tile_pool(name="grp", bufs=1))
    sc = ctx.enter_context(tc.tile_pool(name="sc", bufs=2))
    # four independent 2-bank PSUM rings so concurrent streams don't
    # serialize through a shared rotation
    ps = ctx.enter_context(tc.tile_pool(name="ps", bufs=2, space="PSUM"))
    psi = ctx.enter_context(tc.tile_pool(name="psi", bufs=2, space="PSUM"))
    psf = ctx.enter_context(tc.tile_pool(name="psf", bufs=2, space="PSUM"))
    ps2 = ctx.enter_context(tc.tile_pool(name="ps2", bufs=2, space="PSUM"))

    ispec = input_specs()

    def wtile(name, engine=None):
        shape, dt = ispec[name]
        t = wb.tile(list(shape), BF16 if dt != np.float32 else F32,
                    tag=name, name=name)
        (engine or nc.sync).dma_start(t, ins[name])
        return t

    # xt first: LN1 stats are the kernel's entry dependency
    xt = [[big.tile([128, L], F32, tag=f"bigf{b * T + t}",
                    name=f"bigf{b * T + t}") for t in range(T)]
          for b in range(B)]
    for b in range(B):
        for t in range(T):
            nc.sync.dma_start(xt[b][t], ins["xt"][b, t])

    w_stF_h = wtile("w_stF_h")
    w_bc1w = wtile("w_bc1w")
    vb = wtile("vb")
    w_ipx = wtile("w_ipx")
    w_ipz = wtile("w_ipz")
    w_stG = wtile("w_stG")
    w_bci = wtile("w_bci")
    w_on = wtile("w_on")
    w_dtx = wtile("w_dtx")
    w_bb = wtile("w_bb")
    w_cc = wtile("w_cc")
    w_op = wtile("w_op")
    w_fcs = wtile("w_fcs")
    w_pj = wtile("w_pj", nc.gpsimd)
    w_f2 = wtile("w_f2", nc.gpsimd)

    def V(name):
        i = VB_IDX[name]
        return vb[:, i:i + 1]

    epsv = wb.tile([128, 1], F32, tag="epsv", name="epsv")
    nc.vector.memset(epsv, EPS)

    # main chunking: 448 cols (8 pixel rows), one PSUM bank per tile
    MCK = 448
    NM = L // MCK            # 7

    def mcs(ck):
        return slice(ck * MCK, (ck + 1) * MCK)

    def cs(ck):
        return slice(ck * CK, (ck + 1) * CK)

    def ppt(parts=128, pool=None):
        return (pool or ps).tile([parts, MCK], F32, tag="pp", name="pp")

    def hv(ap):
        return ap

    def mmsplit(out_ps, lhsT, rhs, start=True, stop=True,
                tile_position=None):
        nc.tensor.matmul(out_ps, lhsT, rhs, start=start, stop=stop,
                         tile_position=tile_position)

    xn = [[big.tile([128, L], BF16, tag=f"xn{b * T + t}",
                    name=f"xn{b * T + t}") for t in range(T)]
          for b in range(B)]

    # ---- stats finisher: arena [:,0]=m -> m*rstd ; [:,1]=E[x^2] -> rstd
    # chunked so downstream apply-matmuls start before the whole row is done
    # same-function chunks batched into blocks: every Ln<->Exp alternation
    # costs a 1.28us ACT_TABLE_LOAD (walrus loads single-anchor sets)
    SF = 784
    def stats_finish(ar):
        m = ar[:, 0]
        q = ar[:, 1]
        for ck in range(L // SF):
            s = slice(ck * SF, (ck + 1) * SF)
            t = sc.tile([2, SF], F32, tag="cf", bufs=2)
            nc.vector.scalar_tensor_tensor(t, m[:, s], -1.0, m[:, s],
                                           OP.mult, OP.mult)
            nc.vector.tensor_add(q[:, s], t, q[:, s])
            nc.scalar.activation(q[:, s], q[:, s], AF.Ln, bias=epsv[0:2])
        for ck in range(L // SF):
            s = slice(ck * SF, (ck + 1) * SF)
            nc.scalar.activation(q[:, s], q[:, s], AF.Exp, scale=-0.5)
            nc.vector.tensor_mul(m[:, s], m[:, s], q[:, s])
        return q, m

    # ---- LN stats helper -> (rstd, m*rstd) [2, L]; 448 chunks (1-bank pp2)
    def ln_stats(tiles, is_f32):
        ar = big.tile([2, 2, L], BF16, tag="st_ar", name="st_ar")
        for ck in range(NCK):
            mps = ps2.tile([2, CK], F32, tag="pp2")
            sps = ps2.tile([2, CK], F32, tag="pp2")
            n = len(tiles)
            for i, (tl, b) in enumerate(tiles):
                lw = w_stF_h[:, 2 * b:2 * b + 2]
                rr = tl[:, cs(ck)]
                if is_f32:
                    xb = sc.tile([128, CK], BF16, tag="c1", bufs=3)
                    # split casts ACT/DVE: gpsimd is 2.7x slower per op and
                    # serializes the stats matmuls behind its FIFO
                    if i % 2 == 0:
                        nc.vector.tensor_copy(xb, rr)
                    else:
                        nc.scalar.copy(xb, rr)
                    rr = xb
                sq = sc.tile([128, CK], BF16, tag="c1", bufs=3)
                nc.scalar.activation(sq, tl[:, cs(ck)], AF.Square)
                nc.tensor.matmul(mps, lw, rr, start=(i == 0), stop=(i == n - 1))
                nc.tensor.matmul(sps, lw, sq, start=(i == 0), stop=(i == n - 1))
            nc.vector.tensor_copy(ar[:, 0, cs(ck)], mps)
            nc.scalar.copy(ar[:, 1, cs(ck)], sps)
        return stats_finish(ar)

    def ln_apply(rstd, mr, pairs, lw, bvec=None):
        """each (src, dst): dst = (src - m)*rstd [+b via bvec]; one shared
        broadcast pair per chunk"""
        for ck in range(NM):
            rw = ppt()
            mmsplit(rw, lw, rstd[:, mcs(ck)])
            mw = ppt()
            mmsplit(mw, lw, mr[:, mcs(ck)])
            for src, dst in pairs:
                t1 = hv(sc.tile([128, MCK], F32, tag="c1", name="c1", bufs=3))
                nc.vector.tensor_mul(t1, hv(src[:, mcs(ck)]), rw)
                if bvec is not None:
                    nc.vector.scalar_tensor_tensor(hv(dst[:, mcs(ck)]), t1,
                                                   bvec, mw, OP.add, OP.subtract)
                else:
                    nc.vector.tensor_sub(hv(dst[:, mcs(ck)]), t1, mw)

    # ======== LN1(x) -> xn ========
    r1, m1 = ln_stats([(xt[b][t], b) for b in range(B) for t in range(T)], True)
    for b in range(B):
        for t in range(T):
            lw = w_bc1w[:, (b * T + t) * 128:(b * T + t + 1) * 128]
            ln_apply(r1, m1, [(xt[b][t], xn[b][t])], lw, V(f"n1b{t}"))

    # ======== gate ========
    zs = [[sc.tile([128, 1], BF16, tag=f"zs{b * T + t}", bufs=1,
                   name=f"zs{b * T + t}") for t in range(T)] for b in range(B)]
    gate = [[sc.tile([128, 1], F32, tag=f"gate{b * T + t}", bufs=1,
                     name=f"gate{b * T + t}") for t in range(T)] for b in range(B)]
    for b in range(B):
        for t in range(T):
            with nc.allow_low_precision("bf16 z-sum feeds sigmoid gate"):
                nc.vector.tensor_reduce(zs[b][t], xn[b][t], axis=AX.X, op=OP.add)
    for b in range(B):
        for t in range(T):
            gp = ps2.tile([128, 1], F32, tag="pp2")
            for kt in range(T):
                lw = w_fcs[:, (kt * T + t) * 128:(kt * T + t + 1) * 128]
                nc.tensor.matmul(gp, lw, zs[b][kt],
                                 start=(kt == 0), stop=(kt == T - 1))
            # gate' = 1 + tanh(v/2) = 2*sigmoid(v); the 0.5 is folded into w_op
            nc.scalar.activation(gate[b][t], gp, AF.Tanh,
                                 bias=V(f"fcb{t}"), scale=0.5)
            nc.vector.tensor_scalar_add(gate[b][t], gate[b][t], 1.0)

    # ======== ss2d groups -> ym (pair-interleaved, 784 chunks) ========
    ym = [[big.tile([128, L], BF16, tag=f"bigG{b * T + t}",
                    name=f"bigG{b * T + t}") for t in range(T)]
          for b in range(B)]

    def so_ap(tl, ck, colmajor):
        if not colmajor:
            return tl[:, ck * MCK:(ck + 1) * MCK]
        return bass.AP(tensor=tl.tensor, offset=tl.offset + 8 * ck,
                       ap=[tl.ap[0], [1, 8], [56, 56]])

    padz = [grp.tile([128, LP], BF16, tag=f"padb{j}", name=f"padb{j}")
            for j in range(2)]
    for p_ in padz:
        nc.vector.memset(p_, 0.0)

    ABM_SLOTS = {0: ("bigf1", "bigf2"), 1: ("bigf3", "bigf0")}
    U, SZ, DT, XD, AT, BM, Y = {}, {}, {}, {}, {}, {}, {}
    CVD, STT = {}, {}

    def conv_stage(g):
        colm = g >= 2
        R0 = (g % 2) * 64
        padt = padz[g % 2]
        CVD[g] = grp.tile([128, 9 * 128], BF16, tag=f"cvd{g % 2}",
                          name=f"cvd{g % 2}", bufs=1)
        nc.sync.dma_start(CVD[g], ins["w_cv"][:, g * 9 * 128:(g + 1) * 9 * 128])
        U[g] = grp.tile([128, L], BF16, tag="ub", bufs=2, name=f"u{g}")
        SZ[g] = grp.tile([128, L], BF16, tag="szb", bufs=2, name=f"sz{g}")
        for ck in range(NM):
            xcp = ppt(pool=psi)
            zp = ppt(pool=psi)
            for b in range(B):
                lx = w_ipx[R0:R0 + 64, g * 128 + b * 64:g * 128 + (b + 1) * 64]
                lz = w_ipz[R0:R0 + 64, g * 128 + b * 64:g * 128 + (b + 1) * 64]
                rr = xn[b][g // 2][R0:R0 + 64, mcs(ck)]
                nc.tensor.matmul(xcp[b * 64:(b + 1) * 64], lx, rr,
                                 start=True, stop=True,
                                 tile_position=(R0, b * 64))
                nc.tensor.matmul(zp[b * 64:(b + 1) * 64], lz, rr,
                                 start=True, stop=True,
                                 tile_position=(R0, b * 64))
            dst = bass.AP(tensor=padt.tensor,
                          offset=padt.offset + (1 + 8 * ck) * PW + 1,
                          ap=[padt.ap[0], [PW, 8], [1, 56]])
            nc.vector.tensor_copy(dst, xcp)
            nc.scalar.copy(so_ap(SZ[g], ck, colm), zp)
        for ck in range(NM):
            cvp = ppt(pool=psf)
            for k in range(9):
                dy, dx = k // 3, k % 3
                lhs = CVD[g][:, k * 128:(k + 1) * 128]
                rhs_ = bass.AP(
                    tensor=padt.tensor,
                    offset=padt.offset + (8 * ck + dy) * PW + dx,
                    ap=[padt.ap[0], [PW, 8], [1, 56]])
                nc.tensor.matmul(cvp, lhs, rhs_,
                                 start=(k == 0), stop=(k == 8))
            nc.scalar.copy(so_ap(U[g], ck, colm), cvp)

    def comp_stage(g):
        # silus here (not in conv_stage) so the ACT table set alternates
        # exactly once per group: [silu] then [exp/ln] blocks
        nc.scalar.activation(SZ[g], SZ[g], AF.Silu)
        nc.scalar.activation(U[g], U[g], AF.Silu, bias=V(f"cvb{g}"))
        DT[g] = grp.tile([128, L], BF16, tag="dtb", bufs=2, name=f"dt{g}")
        e1f = grp.tile([128, L], BF16, tag="e1f", bufs=1, name=f"e1f{g}")
        for ck in range(NM):
            dtp = ppt()
            mmsplit(dtp, w_dtx[:, g * 128:(g + 1) * 128], U[g][:, mcs(ck)])
            nc.scalar.activation(e1f[:, mcs(ck)], dtp, AF.Exp, bias=V(f"dtb{g}"))
        for ck in range(NM):
            nc.scalar.activation(DT[g][:, mcs(ck)], e1f[:, mcs(ck)],
                                 AF.Ln, bias=1.0)
        sa, sb = ABM_SLOTS[g % 2]
        AT[g] = big.tile([128, L], BF16, tag=sa, name=f"a{g}")
        BM[g] = big.tile([128, L], BF16, tag=sb, name=f"bm{g}")
        nc.scalar.activation(AT[g], DT[g], AF.Exp, scale=V(f"A{g}"))
        for ck in range(NM):
            bsp = ppt()
            mmsplit(bsp, w_bb[:, g * 128:(g + 1) * 128], U[g][:, mcs(ck)])
            t1 = hv(sc.tile([128, MCK], F32, tag="c1", name="c1", bufs=3))
            nc.vector.tensor_mul(t1, hv(DT[g][:, mcs(ck)]), bsp)
            nc.gpsimd.tensor_mul(hv(BM[g][:, mcs(ck)]), t1,
                                 hv(U[g][:, mcs(ck)]))

    def scan_stage(g):
        if (g % 2) == 1:
            nc.vector.tensor_tensor_scan(BM[g][:, ::-1], AT[g][:, ::-1],
                                         BM[g][:, ::-1], 0.0, OP.mult, OP.add)
        else:
            nc.vector.tensor_tensor_scan(BM[g], AT[g], BM[g],
                                         0.0, OP.mult, OP.add)

    def post_stage(g):
        colm = g >= 2
        R0 = (g % 2) * 64
        Y[g] = grp.tile([128, L], BF16, tag="dtb", bufs=2, name=f"y{g}")
        for ck in range(NM):
            csp = ppt()
            mmsplit(csp, w_cc[:, g * 128:(g + 1) * 128], U[g][:, mcs(ck)])
            t1 = hv(sc.tile([128, MCK], F32, tag="c1", name="c1", bufs=3))
            nc.vector.tensor_mul(t1, hv(BM[g][:, mcs(ck)]), csp)
            nc.vector.scalar_tensor_tensor(hv(Y[g][:, mcs(ck)]),
                                           hv(U[g][:, mcs(ck)]),
                                           V(f"Dp{g}"), t1, OP.mult, OP.add)
        arn = big.tile([2, 2, L], BF16,
                       tag=("st_ar" if g == 0 else
                            "bigf1" if g == 2 else "bigf3"),
                       name=f"st_g{g}")
        for ck in range(NCK):
            ysq = sc.tile([128, CK], BF16, tag="c1", bufs=3)
            nc.scalar.activation(ysq, Y[g][:, cs(ck)], AF.Square)
            mps = ps2.tile([2, CK], F32, tag="pp2")
            sps = ps2.tile([2, CK], F32, tag="pp2")
            nc.tensor.matmul(mps, w_stG, Y[g][:, cs(ck)], start=True, stop=True)
            nc.tensor.matmul(sps, w_stG, ysq, start=True, stop=True)
            nc.vector.tensor_copy(arn[:, 0, cs(ck)], mps)
            nc.scalar.copy(arn[:, 1, cs(ck)], sps)
        rstd, mr = stats_finish(arn)
        lw_on = w_on[:, g * 128:(g + 1) * 128]
        for ck in range(NM):
            rw = ppt(pool=psi)
            mmsplit(rw, lw_on, rstd[:, mcs(ck)])
            mw = ppt(pool=psi)
            mmsplit(mw, lw_on, mr[:, mcs(ck)])
            t1 = hv(sc.tile([128, MCK], F32, tag="c1", name="c1", bufs=3))
            nc.vector.tensor_mul(t1, hv(Y[g][:, mcs(ck)]), rw)
            nc.vector.scalar_tensor_tensor(t1, t1, V(f"onb{g}"), mw,
                                           OP.add, OP.subtract)
            yh = sc.tile([128, MCK], BF16, tag="c3", bufs=2)
            nc.gpsimd.tensor_mul(hv(yh), t1, hv(SZ[g][:, mcs(ck)]))
            for b in range(B):
                # psi ring: keeps the finisher-blocked out_proj off the pp
                # ring so comp(g+1) matmuls run during the finisher wait
                op_ps = ppt(pool=psi)
                lhs = w_op[b * 64:(b + 1) * 64, g * 64:(g + 1) * 64]
                nc.tensor.matmul(op_ps[R0:R0 + 64], lhs,
                                 yh[b * 64:(b + 1) * 64],
                                 start=True, stop=True,
                                 tile_position=(b * 64, R0))
                ymt = ym[b][g // 2]
                xnt = xn[b][g // 2]
                if colm:
                    dst = bass.AP(tensor=ymt.tensor,
                                  offset=ymt.offset + 8 * ck,
                                  ap=[[ymt.ap[0][0], 128], [1, 8],
                                      [56, 56]])[R0:R0 + 64]
                    xnsrc = bass.AP(tensor=xnt.tensor,
                                    offset=xnt.offset + 8 * ck,
                                    ap=[[xnt.ap[0][0], 128], [1, 8],
                                        [56, 56]])[R0:R0 + 64]
                else:
                    dst = hv(ymt[R0:R0 + 64, mcs(ck)])
                    xnsrc = hv(xnt[R0:R0 + 64, mcs(ck)])
                nc.vector.scalar_tensor_tensor(
                    dst, op_ps[R0:R0 + 64], gate[b][g // 2][R0:R0 + 64],
                    xnsrc, OP.mult, OP.mult)

    # LNym stats split into two passes: the t-column finished by groups 0/1
    # is reduced early (its matmuls fill the scan(2)/scan(3) PE stalls); the
    # t=1 pass accumulates into the same arena after group 3
    ymar_h = {}

    def ym_stats_pass(tcol, first):
        if first:
            # allocated lazily: must enter the st_ar ring AFTER arn(0)
            ymar_h["t"] = big.tile([2, 2, L], BF16, tag="st_ar", name="ymar")
        ymar = ymar_h["t"]
        for ck in range(NCK):
            mps = ps2.tile([2, CK], F32, tag="pp2")
            sps = ps2.tile([2, CK], F32, tag="pp2")
            for i in range(B):
                tl = ym[i][tcol]
                lw = w_stF_h[:, 2 * i:2 * i + 2]
                sq = sc.tile([128, CK], BF16, tag="c1", bufs=3)
                nc.scalar.activation(sq, tl[:, cs(ck)], AF.Square)
                nc.tensor.matmul(mps, lw, tl[:, cs(ck)],
                                 start=(i == 0), stop=(i == B - 1))
                nc.tensor.matmul(sps, lw, sq,
                                 start=(i == 0), stop=(i == B - 1))
            if first:
                nc.vector.tensor_copy(ymar[:, 0, cs(ck)], mps)
                nc.scalar.copy(ymar[:, 1, cs(ck)], sps)
            else:
                nc.vector.tensor_add(ymar[:, 0, cs(ck)],
                                     ymar[:, 0, cs(ck)], mps)
                nc.vector.tensor_add(ymar[:, 1, cs(ck)],
                                     ymar[:, 1, cs(ck)], sps)

    # software-pipelined group schedule: conv work of group g+1 is emitted
    # before the scan of group g so PE (and the DVE FIFO) have dense work
    # while the 6.7us scan runs
    conv_stage(0)
    for g in range(G):
        comp_stage(g)
        if g + 1 < G:
            conv_stage(g + 1)
        scan_stage(g)
        post_stage(g)
        if g == 1:
            ym_stats_pass(0, True)

    # ======== LN1(ym) in-place -> ymhat; proj; x2 = xt + proj + b ========
    # reload x in slot-death order (bigf1 frees at scan(2), bigf2 at post(2),
    # bigf3 at scan(3), bigf0 at post(3)) so the sync DMA FIFO never
    # head-of-line blocks on the last-freed slot
    for b, t in ((0, 1), (1, 0), (1, 1), (0, 0)):
        xt[b][t] = big.tile([128, L], F32, tag=f"bigf{b * T + t}",
                            name=f"xt2_{b * T + t}")
        nc.sync.dma_start(xt[b][t], ins["xt"][b, t])
    ym_stats_pass(1, False)
    rym, mym = stats_finish(ymar_h["t"])
    for b in range(B):
        lw = w_bci[:, b * 128:(b + 1) * 128]
        ln_apply(rym, mym, [(ym[b][t], ym[b][t]) for t in range(T)], lw)
    for b in range(B):
        for t in range(T):
            for ck in range(NM):
                # psi ring (idle here): keeps proj off the pp ring so it
                # pipelines per-chunk with the LNym apply instead of
                # queueing behind all of its slot allocations
                xp = ppt(pool=psi)
                for kt in range(T):
                    lhs = w_pj[:, (t * T + kt) * 128:(t * T + kt + 1) * 128]
                    mmsplit(xp, lhs, ym[b][kt][:, mcs(ck)],
                            start=(kt == 0), stop=(kt == T - 1))
                nc.vector.scalar_tensor_tensor(
                    hv(xt[b][t][:, mcs(ck)]), xp, V(f"pjb{t}"),
                    hv(xt[b][t][:, mcs(ck)]), OP.add, OP.add)

    # ======== LN2 -> xhat2 (xn slots); spill x2 to DRAM ========
    r2, m2 = ln_stats([(xt[b][t], b) for b in range(B) for t in range(T)], True)
    xh2 = [[big.tile([128, L], BF16, tag=f"xn{b * T + t}",
                     name=f"xh2_{b * T + t}") for t in range(T)]
           for b in range(B)]
    for b in range(B):
        lw = w_bci[:, b * 128:(b + 1) * 128]
        ln_apply(r2, m2, [(xt[b][t], xh2[b][t]) for t in range(T)], lw)

    # ======== MLP ======== (x2 stays resident in the bigf slots; gel lives
    # in the dead ym/U/SZ slots, so no DRAM spill round-trip is needed)
    for b in range(B):
        gels = ([big.tile([128, L], BF16, tag=f"bigG{j}", name=f"gel{b}_{j}")
                 for j in range(4)]
                + [grp.tile([128, L], BF16, tag="ub", bufs=2,
                            name=f"gelu{b}_{j}") for j in range(2)]
                + [grp.tile([128, L], BF16, tag="szb", bufs=2,
                            name=f"gelz{b}_{j}") for j in range(2)])
        for s in range(HS):
            f1s = grp.tile([128, 2 * 128], BF16, tag=f"f1s{s % 2}",
                           name=f"f1s{s % 2}", bufs=1)
            nc.gpsimd.dma_start(
                f1s[:, 0:128], ins["w_f1"][:, (0 * HS + s) * 128:(0 * HS + s + 1) * 128])
            nc.gpsimd.dma_start(
                f1s[:, 128:256], ins["w_f1"][:, (1 * HS + s) * 128:(1 * HS + s + 1) * 128])
            cvhd = grp.tile([128, 9 * 128], BF16, tag=f"cvd{s % 2}",
                            name=f"cvhd{s % 2}", bufs=1)
            nc.gpsimd.dma_start(cvhd,
                                ins["w_cvh"][:, s * 9 * 128:(s + 1) * 9 * 128])
            padt = padz[s % 2]
            for ck in range(NM):
                hp = ppt()
                for kt in range(T):
                    lhs = f1s[:, kt * 128:(kt + 1) * 128]
                    mmsplit(hp, lhs, xh2[b][kt][:, mcs(ck)],
                            start=(kt == 0), stop=(kt == T - 1))
                dst = bass.AP(tensor=padt.tensor,
                              offset=padt.offset + (1 + 8 * ck) * PW + 1,
                              ap=[padt.ap[0], [PW, 8], [1, 56]])
                nc.vector.tensor_copy(dst, hp)
            for ck in range(NM):
                cvp = ppt(pool=psf if ck % 2 == 0 else psi)
                for k in range(9):
                    dy, dx = k // 3, k % 3
                    lhs = cvhd[:, k * 128:(k + 1) * 128]
                    rhs_ = bass.AP(
                        tensor=padt.tensor,
                        offset=padt.offset + (8 * ck + dy) * PW + dx,
                        ap=[padt.ap[0], [PW, 8], [1, 56]])
                    nc.tensor.matmul(cvp, lhs, rhs_,
                                     start=(k == 0), stop=(k == 8))
                nc.scalar.activation(hv(gels[s][:, mcs(ck)]), cvp,
                                     AF.Gelu, bias=V(f"dwb{s}"))
        for t in range(T):
            for ck in range(NM):
                xp = ppt()
                for s in range(HS):
                    lhs = w_f2[:, (s * T + t) * 128:(s * T + t + 1) * 128]
                    mmsplit(xp, lhs, gels[s][:, mcs(ck)],
                            start=(s == 0), stop=(s == HS - 1))
                x3 = sc.tile([128, MCK], F32, tag="c1", bufs=3)
                nc.vector.scalar_tensor_tensor(
                    hv(x3), xp, V(f"f2b{t}"), hv(xt[b][t][:, mcs(ck)]),
                    OP.add, OP.add)
                nc.sync.dma_start(outs["out"][b, t, :, mcs(ck)], x3)


# =================================================================
# Runner: full-input kernel() entry point. Shards batch over 8 cores,
# compiles the Bass module once, runs SPMD via PJRT/axon, gathers.
# =================================================================
import concourse.bacc as bacc
import concourse.bass_utils as bass_utils

N_CORES = 8
_CACHE = {}


def _build_nc():
    if "nc" in _CACHE:
        return _CACHE["nc"]
    nc = bacc.Bacc("TRN2", debug=False, num_devices=N_CORES)
    ispec = input_specs()
    ins = {}
    for name, (shape, dt) in ispec.items():
        mdt = F32 if dt == np.float32 else BF16
        ins[name] = nc.dram_tensor(name, shape, mdt, kind="ExternalInput").ap()
    out = nc.dram_tensor("out", (B, T, 128, L), F32, kind="ExternalOutput").ap()
    import concourse.tile as _tile
    with _tile.TileContext(nc) as tc:
        with ExitStack() as ctx:
            body(ctx, tc, {"out": out}, ins)
    nc.compile()
    _CACHE["nc"] = nc
    return nc


def make_in_maps(inputs):
    x = np.asarray(inputs["x"], dtype=np.float32)
    return [host_prep(x[c * B:(c + 1) * B], inputs) for c in range(N_CORES)]


def kernel(**inputs):
    nc = _build_nc()
    in_maps = make_in_maps(inputs)
    res = bass_utils.run_bass_kernel_spmd(nc, in_maps,
                                          core_ids=list(range(N_CORES)))
    outs = []
    for c in range(N_CORES):
        o = res.results[c]["out"].reshape(B, C, L).transpose(0, 2, 1)
        outs.append(o)
    return np.ascontiguousarray(np.concatenate(outs, axis=0))



# revision 40
# speedup vs baseline: 1.0061x; 1.0061x over previous
"""GroupMamba block kernel for TRN2 — per-core body + host weight prep.

Per-core work: 2 batches of the (16, 3136, 256) problem. Layout is
channel-partition: activations live as [128 ch, L=3136] tiles, one per
(batch, ctile). All cross-partition ops (LN stats, broadcasts, dwconv,
projections) go through the TensorEngine with host-built block matrices.
The Mamba recurrence is a single tensor_tensor_scan per group.
"""
import numpy as np
from contextlib import ExitStack

import concourse.bass as bass
import concourse.tile as tile  # noqa: F401
from concourse import mybir

F32 = mybir.dt.float32
F32R = mybir.dt.float32r
BF16 = mybir.dt.bfloat16
AF = mybir.ActivationFunctionType
OP = mybir.AluOpType
AX = mybir.AxisListType

B = 2          # batches per core
T = 2          # ctiles (256 = 2*128)
G = 4          # ss2d groups
Cg = 64
C = 256
H = W = 56
L = H * W      # 3136
HID = 1024
HS = 8         # hidden slices of 128
CK = 448       # L-chunk (8 pixel rows)
NCK = L // CK  # 7
PW = 64        # padded row stride
PR = 58        # padded rows
LP = PR * PW   # 3712
EPS = 1e-5

VB_NAMES = ([f"A{g}" for g in range(G)] + [f"dtb{g}" for g in range(G)]
            + [f"cvb{g}" for g in range(G)] + [f"Dp{g}" for g in range(G)]
            + [f"onb{g}" for g in range(G)] + [f"n1b{t}" for t in range(T)]
            + [f"pjb{t}" for t in range(T)] + [f"f1b{s}" for s in range(HS)]
            + [f"f2b{t}" for t in range(T)] + [f"dwb{s}" for s in range(HS)]
            + [f"fcb{t}" for t in range(T)])
VB_IDX = {n: i for i, n in enumerate(VB_NAMES)}


def f32r(ap):
    return ap.bitcast(F32R)


# ---------------------------------------------------------------- host prep
def host_prep(x2b, w):
    """x2b: (2, 3136, 256) f32 shard; w: dict of full weights.
    Returns the per-core device input map (numpy arrays)."""
    import ml_dtypes
    bf = ml_dtypes.bfloat16
    N = np.float32

    def bfar(a):
        return np.ascontiguousarray(np.asarray(a, dtype=np.float32)).astype(bf)

    out = {}
    xt = np.asarray(x2b, dtype=N).transpose(0, 2, 1).reshape(B, T, 128, L)
    out["xt"] = np.ascontiguousarray(xt)

    n1w = np.asarray(w["norm1_w"], N); n1b = np.asarray(w["norm1_b"], N)
    n2w = np.asarray(w["norm2_w"], N); n2b = np.asarray(w["norm2_b"], N)
    skip = float(np.asarray(w["skip_scale"]).reshape(-1)[0])

    stF = np.zeros((128, 4), N)
    stF[:, 0] = 1.0 / C
    stF[:, 3] = 1.0 / C
    out["w_stF_f"] = stF
    out["w_stF_h"] = bfar(stF)
    stG = np.zeros((128, 2), N)
    stG[:64, 0] = 1.0 / Cg
    stG[64:, 1] = 1.0 / Cg
    out["w_stG"] = bfar(stG)

    bc1w = np.zeros((2, B * T * 128), N)
    for b in range(B):
        for t in range(T):
            bc1w[b, (b * T + t) * 128:(b * T + t + 1) * 128] = \
                n1w[t * 128:(t + 1) * 128]
    out["w_bc1w"] = bfar(bc1w)
    bci = np.zeros((2, B * 128), N)
    bci[0, :128] = 1.0
    bci[1, 128:] = 1.0
    out["w_bci"] = bfar(bci)
    bon = np.zeros((2, G * 128), N)
    onw = np.asarray(w["out_norm_w"], N)
    for g in range(G):
        bon[0, g * 128:g * 128 + 64] = onw[g]
        bon[1, g * 128 + 64:(g + 1) * 128] = onw[g]
    out["w_on"] = bfar(bon)

    ipw = np.asarray(w["in_proj_w"], N)
    ipx = np.zeros((128, G, 128), N)
    ipz = np.zeros((128, G, 128), N)
    for g in range(G):
        R = (g % 2) * 64
        bx = ipw[g][:64].T
        bz = ipw[g][64:].T
        ipx[R:R + 64, g, 0:64] = bx
        ipx[R:R + 64, g, 64:128] = bx
        ipz[R:R + 64, g, 0:64] = bz
        ipz[R:R + 64, g, 64:128] = bz
    out["w_ipx"] = bfar(ipx.reshape(128, G * 128))
    out["w_ipz"] = bfar(ipz.reshape(128, G * 128))

    cw = np.asarray(w["conv_w"], N)
    cvd = np.zeros((128, G, 9, 128), N)
    for g in range(G):
        for k in range(9):
            v = cw[g, :, k // 3, k % 3]
            cvd[np.arange(128), g, k, np.arange(128)] = np.concatenate([v, v])
    out["w_cv"] = bfar(cvd.reshape(128, G * 9 * 128))

    dww = np.asarray(w["dw_w"], N)
    cvh = np.zeros((128, HS, 9, 128), N)
    for s in range(HS):
        for k in range(9):
            v = dww[s * 128:(s + 1) * 128, k // 3, k % 3]
            cvh[np.arange(128), s, k, np.arange(128)] = v
    out["w_cvh"] = bfar(cvh.reshape(128, HS * 9 * 128))

    # x_proj and dt_proj folded into single per-group [64,64] / rank-1
    # matrices applied directly to u (no XD intermediate on device)
    xpw = np.asarray(w["x_proj_w"], N)
    dtw = np.asarray(w["dt_proj_w"], N)
    dtx = np.zeros((128, G, 128), N)
    bbl = np.zeros((128, G, 128), N)
    ccl = np.zeros((128, G, 128), N)
    for g in range(G):
        blk = (dtw[g] @ xpw[g][:4]).T          # [k, c]
        for b in range(2):
            sl = slice(b * 64, (b + 1) * 64)
            dtx[sl, g, sl] = blk
            bbl[sl, g, sl] = xpw[g][4][:, None]
            ccl[sl, g, sl] = xpw[g][5][:, None]
    out["w_dtx"] = bfar(dtx.reshape(128, G * 128))
    out["w_bb"] = bfar(bbl.reshape(128, G * 128))
    out["w_cc"] = bfar(ccl.reshape(128, G * 128))

    opw = np.asarray(w["out_proj_w"], N)
    opl = np.zeros((128, G, 64), N)
    for g in range(G):
        # extra 0.5: gate is computed as 1+tanh(v/2) = 2*sigmoid(v)
        blk = (opw[g] * skip * 0.5).T
        opl[0:64, g] = blk
        opl[64:128, g] = blk
    out["w_op"] = bfar(opl.reshape(128, G * 64))

    pw = np.asarray(w["proj_w"], N) * n1w[None, :]
    pj = np.zeros((128, T, T, 128), N)
    for t in range(T):
        for kt in range(T):
            pj[:, t, kt, :] = pw[t * 128:(t + 1) * 128,
                                 kt * 128:(kt + 1) * 128].T
    out["w_pj"] = bfar(pj.reshape(128, T * T * 128))
    pjb = np.asarray(w["proj_b"], N) + np.asarray(w["proj_w"], N) @ n1b

    f1w = np.asarray(w["fc1_w"], N) * n2w[None, :]
    f1 = np.zeros((128, T, HS, 128), N)
    for kt in range(T):
        for hs in range(HS):
            f1[:, kt, hs, :] = f1w[hs * 128:(hs + 1) * 128,
                                   kt * 128:(kt + 1) * 128].T
    out["w_f1"] = bfar(f1.reshape(128, T * HS * 128))
    f1b = np.asarray(w["fc1_b"], N) + np.asarray(w["fc1_w"], N) @ n2b

    f2w = np.asarray(w["fc2_w"], N)
    f2 = np.zeros((128, HS, T, 128), N)
    for hs in range(HS):
        for t in range(T):
            f2[:, hs, t, :] = f2w[t * 128:(t + 1) * 128,
                                  hs * 128:(hs + 1) * 128].T
    out["w_f2"] = bfar(f2.reshape(128, HS * T * 128))

    S = np.zeros((C, C), N)
    ca = np.asarray(w["ca_w"], N)
    for i in range(C):
        for d in range(3):
            j = i + d - 1
            if 0 <= j < C:
                S[i, j] += ca[d]
    fcs = (np.asarray(w["fc_w"], N) + S) / float(L)
    fl = np.zeros((128, T, T, 128), N)
    for kt in range(T):
        for t in range(T):
            fl[:, kt, t, :] = fcs[t * 128:(t + 1) * 128,
                                  kt * 128:(kt + 1) * 128].T
    out["w_fcs"] = bfar(fl.reshape(128, T * T * 128))

    cols = {}
    for g in range(G):
        cols[f"A{g}"] = -np.exp(np.asarray(w["A_log"], N)[g][:, 0])
        cols[f"dtb{g}"] = np.asarray(w["dt_proj_b"], N)[g]
        cols[f"cvb{g}"] = np.asarray(w["conv_b"], N)[g]
        cols[f"Dp{g}"] = np.asarray(w["Dp"], N)[g]
        cols[f"onb{g}"] = np.asarray(w["out_norm_b"], N)[g]
    for t in range(T):
        cols[f"n1b{t}"] = n1b[t * 128:(t + 1) * 128]
        cols[f"pjb{t}"] = pjb[t * 128:(t + 1) * 128]
        cols[f"f2b{t}"] = np.asarray(w["fc2_b"], N)[t * 128:(t + 1) * 128]
        # halved: gate uses tanh(v/2) so the bias enters pre-scaled by 0.5
        cols[f"fcb{t}"] = 0.5 * np.asarray(w["fc_b"], N)[t * 128:(t + 1) * 128]
    for s in range(HS):
        cols[f"f1b{s}"] = f1b[s * 128:(s + 1) * 128]
        cols[f"dwb{s}"] = np.asarray(w["dw_b"], N)[s * 128:(s + 1) * 128]
    vbm = np.zeros((128, len(VB_NAMES)), N)
    for n, i in VB_IDX.items():
        c = cols[n]
        vbm[:, i] = np.concatenate([c, c]) if c.shape[0] == 64 else c
    out["vb"] = vbm
    return out


def input_specs():
    """shapes/dtypes of the device inputs (excluding xt)."""
    import ml_dtypes
    bf = ml_dtypes.bfloat16
    N = np.float32
    return {
        "xt": ((B, T, 128, L), N),
        "w_stF_f": ((128, 4), N),
        "w_stF_h": ((128, 4), bf),
        "w_stG": ((128, 2), bf),
        "w_bc1w": ((2, B * T * 128), bf),
        "w_bci": ((2, B * 128), bf),
        "w_on": ((2, G * 128), bf),
        "w_ipx": ((128, G * 128), bf),
        "w_ipz": ((128, G * 128), bf),
        "w_cv": ((128, G * 9 * 128), bf),
        "w_cvh": ((128, HS * 9 * 128), bf),
        "w_dtx": ((128, G * 128), bf),
        "w_bb": ((128, G * 128), bf),
        "w_cc": ((128, G * 128), bf),
        "w_op": ((128, G * 64), bf),
        "w_pj": ((128, T * T * 128), bf),
        "w_f1": ((128, T * HS * 128), bf),
        "w_f2": ((128, HS * T * 128), bf),
        "w_fcs": ((128, T * T * 128), bf),
        "vb": ((128, len(VB_NAMES)), N),
    }


# ------------------------------------------------------------- device body
def body(ctx: ExitStack, tc, outs, ins):
    nc = tc.nc
    wb = ctx.enter_context(tc.tile_pool(name="wb", bufs=1))
    big = ctx.enter_context(tc.tile_pool(name="big", bufs=1))
    grp = ctx.enter_context(tc.tile_pool(name="grp", bufs=1))
    sc = ctx.enter_context(tc.tile_pool(name="sc", bufs=2))
    # four independent 2-bank PSUM rings so concurrent streams don't
    # serialize through a shared rotation
    ps = ctx.enter_context(tc.tile_pool(name="ps", bufs=2, space="PSUM"))
    psi = ctx.enter_context(tc.tile_pool(name="psi", bufs=2, space="PSUM"))
    psf = ctx.enter_context(tc.tile_pool(name="psf", bufs=2, space="PSUM"))
    ps2 = ctx.enter_context(tc.tile_pool(name="ps2", bufs=2, space="PSUM"))

    ispec = input_specs()

    def wtile(name, engine=None):
        shape, dt = ispec[name]
        t = wb.tile(list(shape), BF16 if dt != np.float32 else F32,
                    tag=name, name=name)
        (engine or nc.sync).dma_start(t, ins[name])
        return t

    # xt first: LN1 stats are the kernel's entry dependency
    xt = [[big.tile([128, L], F32, tag=f"bigf{b * T + t}",
                    name=f"bigf{b * T + t}") for t in range(T)]
          for b in range(B)]
    for b in range(B):
        for t in range(T):
            nc.sync.dma_start(xt[b][t], ins["xt"][b, t])

    w_stF_h = wtile("w_stF_h")
    w_bc1w = wtile("w_bc1w")
    vb = wtile("vb")
    w_ipx = wtile("w_ipx")
    w_ipz = wtile("w_ipz")
    w_stG = wtile("w_stG")
    w_bci = wtile("w_bci")
    w_on = wtile("w_on")
    w_dtx = wtile("w_dtx")
    w_bb = wtile("w_bb")
    w_cc = wtile("w_cc")
    w_op = wtile("w_op")
    w_fcs = wtile("w_fcs")
    w_pj = wtile("w_pj", nc.gpsimd)
    w_f2 = wtile("w_f2", nc.gpsimd)

    def V(name):
        i = VB_IDX[name]
        return vb[:, i:i + 1]

    epsv = wb.tile([128, 1], F32, tag="epsv", name="epsv")
    nc.vector.memset(epsv, EPS)

    # main chunking: 448 cols (8 pixel rows), one PSUM bank per tile
    MCK = 448
    NM = L // MCK            # 7

    def mcs(ck):
        return slice(ck * MCK, (ck + 1) * MCK)

    def cs(ck):
        return slice(ck * CK, (ck + 1) * CK)

    def ppt(parts=128, pool=None):
        return (pool or ps).tile([parts, MCK], F32, tag="pp", name="pp")

    def hv(ap):
        return ap

    def mmsplit(out_ps, lhsT, rhs, start=True, stop=True,
                tile_position=None):
        nc.tensor.matmul(out_ps, lhsT, rhs, start=start, stop=stop,
                         tile_position=tile_position)

    xn = [[big.tile([128, L], BF16, tag=f"xn{b * T + t}",
                    name=f"xn{b * T + t}") for t in range(T)]
          for b in range(B)]

    # ---- stats finisher: arena [:,0]=m -> m*rstd ; [:,1]=E[x^2] -> rstd
    # chunked so downstream apply-matmuls start before the whole row is done
    # same-function chunks batched into blocks: every Ln<->Exp alternation
    # costs a 1.28us ACT_TABLE_LOAD (walrus loads single-anchor sets)
    SF = 784
    def stats_finish(ar):
        m = ar[:, 0]
        q = ar[:, 1]
        for ck in range(L // SF):
            s = slice(ck * SF, (ck + 1) * SF)
            t = sc.tile([2, SF], F32, tag="cf", bufs=2)
            nc.vector.scalar_tensor_tensor(t, m[:, s], -1.0, m[:, s],
                                           OP.mult, OP.mult)
            nc.vector.tensor_add(q[:, s], t, q[:, s])
            nc.scalar.activation(q[:, s], q[:, s], AF.Ln, bias=epsv[0:2])
        for ck in range(L // SF):
            s = slice(ck * SF, (ck + 1) * SF)
            nc.scalar.activation(q[:, s], q[:, s], AF.Exp, scale=-0.5)
            nc.vector.tensor_mul(m[:, s], m[:, s], q[:, s])
        return q, m

    # ---- LN stats helper -> (rstd, m*rstd) [2, L]; 448 chunks (1-bank pp2)
    def ln_stats(tiles, is_f32):
        ar = big.tile([2, 2, L], BF16, tag="st_ar", name="st_ar")
        for ck in range(NCK):
            mps = ps2.tile([2, CK], F32, tag="pp2")
            sps = ps2.tile([2, CK], F32, tag="pp2")
            n = len(tiles)
            for i, (tl, b) in enumerate(tiles):
                lw = w_stF_h[:, 2 * b:2 * b + 2]
                rr = tl[:, cs(ck)]
                if is_f32:
                    xb = sc.tile([128, CK], BF16, tag="c1", bufs=3)
                    # split casts ACT/DVE: gpsimd is 2.7x slower per op and
                    # serializes the stats matmuls behind its FIFO
                    if i % 2 == 0:
                        nc.vector.tensor_copy(xb, rr)
                    else:
                        nc.scalar.copy(xb, rr)
                    rr = xb
                sq = sc.tile([128, CK], BF16, tag="c1", bufs=3)
                nc.scalar.activation(sq, tl[:, cs(ck)], AF.Square)
                nc.tensor.matmul(mps, lw, rr, start=(i == 0), stop=(i == n - 1))
                nc.tensor.matmul(sps, lw, sq, start=(i == 0), stop=(i == n - 1))
            nc.vector.tensor_copy(ar[:, 0, cs(ck)], mps)
            nc.scalar.copy(ar[:, 1, cs(ck)], sps)
        return stats_finish(ar)

    def ln_apply(rstd, mr, pairs, lw, bvec=None):
        """each (src, dst): dst = (src - m)*rstd [+b via bvec]; one shared
        broadcast pair per chunk"""
        for ck in range(NM):
            rw = ppt()
            mmsplit(rw, lw, rstd[:, mcs(ck)])
            mw = ppt()
            mmsplit(mw, lw, mr[:, mcs(ck)])
            for src, dst in pairs:
                t1 = hv(sc.tile([128, MCK], F32, tag="c1", name="c1", bufs=3))
                nc.vector.tensor_mul(t1, hv(src[:, mcs(ck)]), rw)
                if bvec is not None:
                    nc.vector.scalar_tensor_tensor(hv(dst[:, mcs(ck)]), t1,
                                                   bvec, mw, OP.add, OP.subtract)
                else:
                    nc.vector.tensor_sub(hv(dst[:, mcs(ck)]), t1, mw)

    # ======== LN1(x) -> xn ========
    r1, m1 = ln_stats([(xt[b][t], b) for b in range(B) for t in range(T)], True)
    for b in range(B):
        for t in range(T):
            lw = w_bc1w[:, (b * T + t) * 128:(b * T + t + 1) * 128]
            ln_apply(r1, m1, [(xt[b][t], xn[b][t])], lw, V(f"n1b{t}"))

    # ======== gate ========
    zs = [[sc.tile([128, 1], BF16, tag=f"zs{b * T + t}", bufs=1,
                   name=f"zs{b * T + t}") for t in range(T)] for b in range(B)]
    gate = [[sc.tile([128, 1], F32, tag=f"gate{b * T + t}", bufs=1,
                     name=f"gate{b * T + t}") for t in range(T)] for b in range(B)]
    for b in range(B):
        for t in range(T):
            with nc.allow_low_precision("bf16 z-sum feeds sigmoid gate"):
                nc.vector.tensor_reduce(zs[b][t], xn[b][t], axis=AX.X, op=OP.add)
    for b in range(B):
        for t in range(T):
            gp = ps2.tile([128, 1], F32, tag="pp2")
            for kt in range(T):
                lw = w_fcs[:, (kt * T + t) * 128:(kt * T + t + 1) * 128]
                nc.tensor.matmul(gp, lw, zs[b][kt],
                                 start=(kt == 0), stop=(kt == T - 1))
            # gate' = 1 + tanh(v/2) = 2*sigmoid(v); the 0.5 is folded into w_op
            nc.scalar.activation(gate[b][t], gp, AF.Tanh,
                                 bias=V(f"fcb{t}"), scale=0.5)
            nc.vector.tensor_scalar_add(gate[b][t], gate[b][t], 1.0)

    # ======== ss2d groups -> ym (pair-interleaved, 784 chunks) ========
    ym = [[big.tile([128, L], BF16, tag=f"bigG{b * T + t}",
                    name=f"bigG{b * T + t}") for t in range(T)]
          for b in range(B)]

    def so_ap(tl, ck, colmajor):
        if not colmajor:
            return tl[:, ck * MCK:(ck + 1) * MCK]
        return bass.AP(tensor=tl.tensor, offset=tl.offset + 8 * ck,
                       ap=[tl.ap[0], [1, 8], [56, 56]])

    padz = [grp.tile([128, LP], BF16, tag=f"padb{j}", name=f"padb{j}")
            for j in range(2)]
    for p_ in padz:
        nc.vector.memset(p_, 0.0)

    ABM_SLOTS = {0: ("bigf1", "bigf2"), 1: ("bigf3", "bigf0")}
    U, SZ, DT, XD, AT, BM, Y = {}, {}, {}, {}, {}, {}, {}
    CVD, STT = {}, {}

    def conv_stage(g):
        colm = g >= 2
        R0 = (g % 2) * 64
        padt = padz[g % 2]
        CVD[g] = grp.tile([128, 9 * 128], BF16, tag=f"cvd{g % 2}",
                          name=f"cvd{g % 2}", bufs=1)
        nc.sync.dma_start(CVD[g], ins["w_cv"][:, g * 9 * 128:(g + 1) * 9 * 128])
        U[g] = grp.tile([128, L], BF16, tag="ub", bufs=2, name=f"u{g}")
        SZ[g] = grp.tile([128, L], BF16, tag="szb", bufs=2, name=f"sz{g}")
        for ck in range(NM):
            xcp = ppt(pool=psi)
            zp = ppt(pool=psi)
            for b in range(B):
                lx = w_ipx[R0:R0 + 64, g * 128 + b * 64:g * 128 + (b + 1) * 64]
                lz = w_ipz[R0:R0 + 64, g * 128 + b * 64:g * 128 + (b + 1) * 64]
                rr = xn[b][g // 2][R0:R0 + 64, mcs(ck)]
                nc.tensor.matmul(xcp[b * 64:(b + 1) * 64], lx, rr,
                                 start=True, stop=True,
                                 tile_position=(R0, b * 64))
                nc.tensor.matmul(zp[b * 64:(b + 1) * 64], lz, rr,
                                 start=True, stop=True,
                                 tile_position=(R0, b * 64))
            dst = bass.AP(tensor=padt.tensor,
                          offset=padt.offset + (1 + 8 * ck) * PW + 1,
                          ap=[padt.ap[0], [PW, 8], [1, 56]])
            nc.vector.tensor_copy(dst, xcp)
            nc.scalar.copy(so_ap(SZ[g], ck, colm), zp)
        for ck in range(NM):
            cvp = ppt(pool=psf)
            for k in range(9):
                dy, dx = k // 3, k % 3
                lhs = CVD[g][:, k * 128:(k + 1) * 128]
                rhs_ = bass.AP(
                    tensor=padt.tensor,
                    offset=padt.offset + (8 * ck + dy) * PW + dx,
                    ap=[padt.ap[0], [PW, 8], [1, 56]])
                nc.tensor.matmul(cvp, lhs, rhs_,
                                 start=(k == 0), stop=(k == 8))
            nc.scalar.copy(so_ap(U[g], ck, colm), cvp)

    def comp_stage(g):
        # silus here (not in conv_stage) so the ACT table set alternates
        # exactly once per group: [silu] then [exp/ln] blocks
        nc.scalar.activation(SZ[g], SZ[g], AF.Silu)
        nc.scalar.activation(U[g], U[g], AF.Silu, bias=V(f"cvb{g}"))
        DT[g] = grp.tile([128, L], BF16, tag="dtb", bufs=2, name=f"dt{g}")
        e1f = grp.tile([128, L], BF16, tag="e1f", bufs=1, name=f"e1f{g}")
        for ck in range(NM):
            dtp = ppt()
            mmsplit(dtp, w_dtx[:, g * 128:(g + 1) * 128], U[g][:, mcs(ck)])
            nc.scalar.activation(e1f[:, mcs(ck)], dtp, AF.Exp, bias=V(f"dtb{g}"))
        for ck in range(NM):
            nc.scalar.activation(DT[g][:, mcs(ck)], e1f[:, mcs(ck)],
                                 AF.Ln, bias=1.0)
        sa, sb = ABM_SLOTS[g % 2]
        AT[g] = big.tile([128, L], BF16, tag=sa, name=f"a{g}")
        BM[g] = big.tile([128, L], BF16, tag=sb, name=f"bm{g}")
        nc.scalar.activation(AT[g], DT[g], AF.Exp, scale=V(f"A{g}"))
        for ck in range(NM):
            bsp = ppt()
            mmsplit(bsp, w_bb[:, g * 128:(g + 1) * 128], U[g][:, mcs(ck)])
            t1 = hv(sc.tile([128, MCK], F32, tag="c1", name="c1", bufs=3))
            nc.vector.tensor_mul(t1, hv(DT[g][:, mcs(ck)]), bsp)
            nc.gpsimd.tensor_mul(hv(BM[g][:, mcs(ck)]), t1,
                                 hv(U[g][:, mcs(ck)]))

    def scan_stage(g):
        if (g % 2) == 1:
            nc.vector.tensor_tensor_scan(BM[g][:, ::-1], AT[g][:, ::-1],
                                         BM[g][:, ::-1], 0.0, OP.mult, OP.add)
        else:
            nc.vector.tensor_tensor_scan(BM[g], AT[g], BM[g],
                                         0.0, OP.mult, OP.add)

    def post_stage(g):
        colm = g >= 2
        R0 = (g % 2) * 64
        Y[g] = grp.tile([128, L], BF16, tag="dtb", bufs=2, name=f"y{g}")
        for ck in range(NM):
            csp = ppt()
            mmsplit(csp, w_cc[:, g * 128:(g + 1) * 128], U[g][:, mcs(ck)])
            t1 = hv(sc.tile([128, MCK], F32, tag="c1", name="c1", bufs=3))
            nc.vector.tensor_mul(t1, hv(BM[g][:, mcs(ck)]), csp)
            nc.vector.scalar_tensor_tensor(hv(Y[g][:, mcs(ck)]),
                                           hv(U[g][:, mcs(ck)]),
                                           V(f"Dp{g}"), t1, OP.mult, OP.add)
        arn = big.tile([2, 2, L], BF16,
                       tag=("st_ar" if g == 0 else
                            "bigf1" if g == 2 else "bigf3"),
                       name=f"st_g{g}")
        for ck in range(NCK):
            ysq = sc.tile([128, CK], BF16, tag="c1", bufs=3)
            nc.scalar.activation(ysq, Y[g][:, cs(ck)], AF.Square)
            mps = ps2.tile([2, CK], F32, tag="pp2")
            sps = ps2.tile([2, CK], F32, tag="pp2")
            nc.tensor.matmul(mps, w_stG, Y[g][:, cs(ck)], start=True, stop=True)
            nc.tensor.matmul(sps, w_stG, ysq, start=True, stop=True)
            nc.vector.tensor_copy(arn[:, 0, cs(ck)], mps)
            nc.scalar.copy(arn[:, 1, cs(ck)], sps)
        rstd, mr = stats_finish(arn)
        lw_on = w_on[:, g * 128:(g + 1) * 128]
        for ck in range(NM):
            rw = ppt(pool=psi)
            mmsplit(rw, lw_on, rstd[:, mcs(ck)])
            mw = ppt(pool=psi)
            mmsplit(mw, lw_on, mr[:, mcs(ck)])
            t1 = hv(sc.tile([128, MCK], F32, tag="c1", name="c1", bufs=3))
            nc.vector.tensor_mul(t1, hv(Y[g][:, mcs(ck)]), rw)
            nc.vector.scalar_tensor_tensor(t1, t1, V(f"onb{g}"), mw,
                                           OP.add, OP.subtract)
            yh = sc.tile([128, MCK], BF16, tag="c3", bufs=2)
            nc.gpsimd.tensor_mul(hv(yh), t1, hv(SZ[g][:, mcs(ck)]))
            for b in range(B):
                op_ps = ppt()
                lhs = w_op[b * 64:(b + 1) * 64, g * 64:(g + 1) * 64]
                nc.tensor.matmul(op_ps[R0:R0 + 64], lhs,
                                 yh[b * 64:(b + 1) * 64],
                                 start=True, stop=True,
                                 tile_position=(b * 64, R0))
                ymt = ym[b][g // 2]
                xnt = xn[b][g // 2]
                if colm:
                    dst = bass.AP(tensor=ymt.tensor,
                                  offset=ymt.offset + 8 * ck,
                                  ap=[[ymt.ap[0][0], 128], [1, 8],
                                      [56, 56]])[R0:R0 + 64]
                    xnsrc = bass.AP(tensor=xnt.tensor,
                                    offset=xnt.offset + 8 * ck,
                                    ap=[[xnt.ap[0][0], 128], [1, 8],
                                        [56, 56]])[R0:R0 + 64]
                else:
                    dst = hv(ymt[R0:R0 + 64, mcs(ck)])
                    xnsrc = hv(xnt[R0:R0 + 64, mcs(ck)])
                nc.vector.scalar_tensor_tensor(
                    dst, op_ps[R0:R0 + 64], gate[b][g // 2][R0:R0 + 64],
                    xnsrc, OP.mult, OP.mult)

    # LNym stats split into two passes: the t-column finished by groups 0/1
    # is reduced early (its matmuls fill the scan(2)/scan(3) PE stalls); the
    # t=1 pass accumulates into the same arena after group 3
    ymar_h = {}

    def ym_stats_pass(tcol, first):
        if first:
            # allocated lazily: must enter the st_ar ring AFTER arn(0)
            ymar_h["t"] = big.tile([2, 2, L], BF16, tag="st_ar", name="ymar")
        ymar = ymar_h["t"]
        for ck in range(NCK):
            mps = ps2.tile([2, CK], F32, tag="pp2")
            sps = ps2.tile([2, CK], F32, tag="pp2")
            for i in range(B):
                tl = ym[i][tcol]
                lw = w_stF_h[:, 2 * i:2 * i + 2]
                sq = sc.tile([128, CK], BF16, tag="c1", bufs=3)
                nc.scalar.activation(sq, tl[:, cs(ck)], AF.Square)
                nc.tensor.matmul(mps, lw, tl[:, cs(ck)],
                                 start=(i == 0), stop=(i == B - 1))
                nc.tensor.matmul(sps, lw, sq,
                                 start=(i == 0), stop=(i == B - 1))
            if first:
                nc.vector.tensor_copy(ymar[:, 0, cs(ck)], mps)
                nc.scalar.copy(ymar[:, 1, cs(ck)], sps)
            else:
                nc.vector.tensor_add(ymar[:, 0, cs(ck)],
                                     ymar[:, 0, cs(ck)], mps)
                nc.vector.tensor_add(ymar[:, 1, cs(ck)],
                                     ymar[:, 1, cs(ck)], sps)

    # software-pipelined group schedule: conv work of group g+1 is emitted
    # before the scan of group g so PE (and the DVE FIFO) have dense work
    # while the 6.7us scan runs
    conv_stage(0)
    for g in range(G):
        comp_stage(g)
        if g + 1 < G:
            conv_stage(g + 1)
        scan_stage(g)
        post_stage(g)
        if g == 1:
            ym_stats_pass(0, True)

    # ======== LN1(ym) in-place -> ymhat; proj; x2 = xt + proj + b ========
    # reload x in slot-death order (bigf1 frees at scan(2), bigf2 at post(2),
    # bigf3 at scan(3), bigf0 at post(3)) so the sync DMA FIFO never
    # head-of-line blocks on the last-freed slot
    for b, t in ((0, 1), (1, 0), (1, 1), (0, 0)):
        xt[b][t] = big.tile([128, L], F32, tag=f"bigf{b * T + t}",
                            name=f"xt2_{b * T + t}")
        nc.sync.dma_start(xt[b][t], ins["xt"][b, t])
    ym_stats_pass(1, False)
    rym, mym = stats_finish(ymar_h["t"])
    for b in range(B):
        lw = w_bci[:, b * 128:(b + 1) * 128]
        ln_apply(rym, mym, [(ym[b][t], ym[b][t]) for t in range(T)], lw)
    for b in range(B):
        for t in range(T):
            for ck in range(NM):
                # psi ring (idle here): keeps proj off the pp ring so it
                # pipelines per-chunk with the LNym apply instead of
                # queueing behind all of its slot allocations
                xp = ppt(pool=psi)
                for kt in range(T):
                    lhs = w_pj[:, (t * T + kt) * 128:(t * T + kt + 1) * 128]
                    mmsplit(xp, lhs, ym[b][kt][:, mcs(ck)],
                            start=(kt == 0), stop=(kt == T - 1))
                nc.vector.scalar_tensor_tensor(
                    hv(xt[b][t][:, mcs(ck)]), xp, V(f"pjb{t}"),
                    hv(xt[b][t][:, mcs(ck)]), OP.add, OP.add)

    # ======== LN2 -> xhat2 (xn slots); spill x2 to DRAM ========
    r2, m2 = ln_stats([(xt[b][t], b) for b in range(B) for t in range(T)], True)
    xh2 = [[big.tile([128, L], BF16, tag=f"xn{b * T + t}",
                     name=f"xh2_{b * T + t}") for t in range(T)]
           for b in range(B)]
    for b in range(B):
        lw = w_bci[:, b * 128:(b + 1) * 128]
        ln_apply(r2, m2, [(xt[b][t], xh2[b][t]) for t in range(T)], lw)

    # ======== MLP ======== (x2 stays resident in the bigf slots; gel lives
    # in the dead ym/U/SZ slots, so no DRAM spill round-trip is needed)
    for b in range(B):
        gels = ([big.tile([128, L], BF16, tag=f"bigG{j}", name=f"gel{b}_{j}")
                 for j in range(4)]
                + [grp.tile([128, L], BF16, tag="ub", bufs=2,
                            name=f"gelu{b}_{j}") for j in range(2)]
                + [grp.tile([128, L], BF16, tag="szb", bufs=2,
                            name=f"gelz{b}_{j}") for j in range(2)])
        for s in range(HS):
            f1s = grp.tile([128, 2 * 128], BF16, tag=f"f1s{s % 2}",
                           name=f"f1s{s % 2}", bufs=1)
            nc.gpsimd.dma_start(
                f1s[:, 0:128], ins["w_f1"][:, (0 * HS + s) * 128:(0 * HS + s + 1) * 128])
            nc.gpsimd.dma_start(
                f1s[:, 128:256], ins["w_f1"][:, (1 * HS + s) * 128:(1 * HS + s + 1) * 128])
            cvhd = grp.tile([128, 9 * 128], BF16, tag=f"cvd{s % 2}",
                            name=f"cvhd{s % 2}", bufs=1)
            nc.gpsimd.dma_start(cvhd,
                                ins["w_cvh"][:, s * 9 * 128:(s + 1) * 9 * 128])
            padt = padz[s % 2]
            for ck in range(NM):
                hp = ppt()
                for kt in range(T):
                    lhs = f1s[:, kt * 128:(kt + 1) * 128]
                    mmsplit(hp, lhs, xh2[b][kt][:, mcs(ck)],
                            start=(kt == 0), stop=(kt == T - 1))
                dst = bass.AP(tensor=padt.tensor,
                              offset=padt.offset + (1 + 8 * ck) * PW + 1,
                              ap=[padt.ap[0], [PW, 8], [1, 56]])
                nc.vector.tensor_copy(dst, hp)
            for ck in range(NM):
                cvp = ppt(pool=psf if ck % 2 == 0 else psi)
                for k in range(9):
                    dy, dx = k // 3, k % 3
                    lhs = cvhd[:, k * 128:(k + 1) * 128]
                    rhs_ = bass.AP(
                        tensor=padt.tensor,
                        offset=padt.offset + (8 * ck + dy) * PW + dx,
                        ap=[padt.ap[0], [PW, 8], [1, 56]])
                    nc.tensor.matmul(cvp, lhs, rhs_,
                                     start=(k == 0), stop=(k == 8))
                nc.scalar.activation(hv(gels[s][:, mcs(ck)]), cvp,
                                     AF.Gelu, bias=V(f"dwb{s}"))
        for t in range(T):
            for ck in range(NM):
                xp = ppt()
                for s in range(HS):
                    lhs = w_f2[:, (s * T + t) * 128:(s * T + t + 1) * 128]
                    mmsplit(xp, lhs, gels[s][:, mcs(ck)],
                            start=(s == 0), stop=(s == HS - 1))
                x3 = sc.tile([128, MCK], F32, tag="c1", bufs=3)
                nc.vector.scalar_tensor_tensor(
                    hv(x3), xp, V(f"f2b{t}"), hv(xt[b][t][:, mcs(ck)]),
                    OP.add, OP.add)
                nc.sync.dma_start(outs["out"][b, t, :, mcs(ck)], x3)


# =================================================================
# Runner: full-input kernel() entry point. Shards batch over 8 cores,
# compiles the Bass module once, runs SPMD via PJRT/axon, gathers.
# =================================================================
import concourse.bacc as bacc
import concourse.bass_utils as bass_utils

N_CORES = 8
_CACHE = {}


def _build_nc():
    if "nc" in _CACHE:
        return _CACHE["nc"]
    nc = bacc.Bacc("TRN2", debug=False, num_devices=N_CORES)
    ispec = input_specs()
    ins = {}
    for name, (shape, dt) in ispec.items():
        mdt = F32 if dt == np.float32 else BF16
        ins[name] = nc.dram_tensor(name, shape, mdt, kind="ExternalInput").ap()
    out = nc.dram_tensor("out", (B, T, 128, L), F32, kind="ExternalOutput").ap()
    import concourse.tile as _tile
    with _tile.TileContext(nc) as tc:
        with ExitStack() as ctx:
            body(ctx, tc, {"out": out}, ins)
    nc.compile()
    _CACHE["nc"] = nc
    return nc


def make_in_maps(inputs):
    x = np.asarray(inputs["x"], dtype=np.float32)
    return [host_prep(x[c * B:(c + 1) * B], inputs) for c in range(N_CORES)]


def kernel(**inputs):
    nc = _build_nc()
    in_maps = make_in_maps(inputs)
    res = bass_utils.run_bass_kernel_spmd(nc, in_maps,
                                          core_ids=list(range(N_CORES)))
    outs = []
    for c in range(N_CORES):
        o = res.results[c]["out"].reshape(B, C, L).transpose(0, 2, 1)
        outs.append(o)
    return np.ascontiguousarray(np.concatenate(outs, axis=0))



# revision 41
# speedup vs baseline: 1.0141x; 1.0080x over previous
"""GroupMamba block kernel for TRN2 — per-core body + host weight prep.

Per-core work: 2 batches of the (16, 3136, 256) problem. Layout is
channel-partition: activations live as [128 ch, L=3136] tiles, one per
(batch, ctile). All cross-partition ops (LN stats, broadcasts, dwconv,
projections) go through the TensorEngine with host-built block matrices.
The Mamba recurrence is a single tensor_tensor_scan per group.
"""
import numpy as np
from contextlib import ExitStack

import concourse.bass as bass
import concourse.tile as tile  # noqa: F401
from concourse import mybir

F32 = mybir.dt.float32
F32R = mybir.dt.float32r
BF16 = mybir.dt.bfloat16
AF = mybir.ActivationFunctionType
OP = mybir.AluOpType
AX = mybir.AxisListType

B = 2          # batches per core
T = 2          # ctiles (256 = 2*128)
G = 4          # ss2d groups
Cg = 64
C = 256
H = W = 56
L = H * W      # 3136
HID = 1024
HS = 8         # hidden slices of 128
CK = 448       # L-chunk (8 pixel rows)
NCK = L // CK  # 7
PW = 64        # padded row stride
PR = 58        # padded rows
LP = PR * PW   # 3712
EPS = 1e-5

VB_NAMES = ([f"A{g}" for g in range(G)] + [f"dtb{g}" for g in range(G)]
            + [f"cvb{g}" for g in range(G)] + [f"Dp{g}" for g in range(G)]
            + [f"onb{g}" for g in range(G)] + [f"n1b{t}" for t in range(T)]
            + [f"pjb{t}" for t in range(T)] + [f"f1b{s}" for s in range(HS)]
            + [f"f2b{t}" for t in range(T)] + [f"dwb{s}" for s in range(HS)]
            + [f"fcb{t}" for t in range(T)])
VB_IDX = {n: i for i, n in enumerate(VB_NAMES)}


def f32r(ap):
    return ap.bitcast(F32R)


# ---------------------------------------------------------------- host prep
def host_prep(x2b, w):
    """x2b: (2, 3136, 256) f32 shard; w: dict of full weights.
    Returns the per-core device input map (numpy arrays)."""
    import ml_dtypes
    bf = ml_dtypes.bfloat16
    N = np.float32

    def bfar(a):
        return np.ascontiguousarray(np.asarray(a, dtype=np.float32)).astype(bf)

    out = {}
    xt = np.asarray(x2b, dtype=N).transpose(0, 2, 1).reshape(B, T, 128, L)
    out["xt"] = np.ascontiguousarray(xt)

    n1w = np.asarray(w["norm1_w"], N); n1b = np.asarray(w["norm1_b"], N)
    n2w = np.asarray(w["norm2_w"], N); n2b = np.asarray(w["norm2_b"], N)
    skip = float(np.asarray(w["skip_scale"]).reshape(-1)[0])

    stF = np.zeros((128, 4), N)
    stF[:, 0] = 1.0 / C
    stF[:, 3] = 1.0 / C
    out["w_stF_f"] = stF
    out["w_stF_h"] = bfar(stF)
    stG = np.zeros((128, 2), N)
    stG[:64, 0] = 1.0 / Cg
    stG[64:, 1] = 1.0 / Cg
    out["w_stG"] = bfar(stG)

    bc1w = np.zeros((2, B * T * 128), N)
    for b in range(B):
        for t in range(T):
            bc1w[b, (b * T + t) * 128:(b * T + t + 1) * 128] = \
                n1w[t * 128:(t + 1) * 128]
    out["w_bc1w"] = bfar(bc1w)
    bci = np.zeros((2, B * 128), N)
    bci[0, :128] = 1.0
    bci[1, 128:] = 1.0
    out["w_bci"] = bfar(bci)
    bon = np.zeros((2, G * 128), N)
    onw = np.asarray(w["out_norm_w"], N)
    for g in range(G):
        bon[0, g * 128:g * 128 + 64] = onw[g]
        bon[1, g * 128 + 64:(g + 1) * 128] = onw[g]
    out["w_on"] = bfar(bon)

    ipw = np.asarray(w["in_proj_w"], N)
    ipx = np.zeros((128, G, 128), N)
    ipz = np.zeros((128, G, 128), N)
    for g in range(G):
        R = (g % 2) * 64
        bx = ipw[g][:64].T
        bz = ipw[g][64:].T
        ipx[R:R + 64, g, 0:64] = bx
        ipx[R:R + 64, g, 64:128] = bx
        ipz[R:R + 64, g, 0:64] = bz
        ipz[R:R + 64, g, 64:128] = bz
    out["w_ipx"] = bfar(ipx.reshape(128, G * 128))
    out["w_ipz"] = bfar(ipz.reshape(128, G * 128))

    cw = np.asarray(w["conv_w"], N)
    cvd = np.zeros((128, G, 9, 128), N)
    for g in range(G):
        for k in range(9):
            v = cw[g, :, k // 3, k % 3]
            cvd[np.arange(128), g, k, np.arange(128)] = np.concatenate([v, v])
    out["w_cv"] = bfar(cvd.reshape(128, G * 9 * 128))

    dww = np.asarray(w["dw_w"], N)
    cvh = np.zeros((128, HS, 9, 128), N)
    for s in range(HS):
        for k in range(9):
            v = dww[s * 128:(s + 1) * 128, k // 3, k % 3]
            cvh[np.arange(128), s, k, np.arange(128)] = v
    out["w_cvh"] = bfar(cvh.reshape(128, HS * 9 * 128))

    # x_proj and dt_proj folded into single per-group [64,64] / rank-1
    # matrices applied directly to u (no XD intermediate on device)
    xpw = np.asarray(w["x_proj_w"], N)
    dtw = np.asarray(w["dt_proj_w"], N)
    dtx = np.zeros((128, G, 128), N)
    bbl = np.zeros((128, G, 128), N)
    ccl = np.zeros((128, G, 128), N)
    for g in range(G):
        blk = (dtw[g] @ xpw[g][:4]).T          # [k, c]
        for b in range(2):
            sl = slice(b * 64, (b + 1) * 64)
            dtx[sl, g, sl] = blk
            bbl[sl, g, sl] = xpw[g][4][:, None]
            ccl[sl, g, sl] = xpw[g][5][:, None]
    out["w_dtx"] = bfar(dtx.reshape(128, G * 128))
    out["w_bb"] = bfar(bbl.reshape(128, G * 128))
    out["w_cc"] = bfar(ccl.reshape(128, G * 128))

    opw = np.asarray(w["out_proj_w"], N)
    opl = np.zeros((128, G, 64), N)
    for g in range(G):
        # extra 0.5: gate is computed as 1+tanh(v/2) = 2*sigmoid(v)
        blk = (opw[g] * skip * 0.5).T
        opl[0:64, g] = blk
        opl[64:128, g] = blk
    out["w_op"] = bfar(opl.reshape(128, G * 64))

    pw = np.asarray(w["proj_w"], N) * n1w[None, :]
    pj = np.zeros((128, T, T, 128), N)
    for t in range(T):
        for kt in range(T):
            pj[:, t, kt, :] = pw[t * 128:(t + 1) * 128,
                                 kt * 128:(kt + 1) * 128].T
    out["w_pj"] = bfar(pj.reshape(128, T * T * 128))
    pjb = np.asarray(w["proj_b"], N) + np.asarray(w["proj_w"], N) @ n1b

    f1w = np.asarray(w["fc1_w"], N) * n2w[None, :]
    f1 = np.zeros((128, T, HS, 128), N)
    for kt in range(T):
        for hs in range(HS):
            f1[:, kt, hs, :] = f1w[hs * 128:(hs + 1) * 128,
                                   kt * 128:(kt + 1) * 128].T
    out["w_f1"] = bfar(f1.reshape(128, T * HS * 128))
    f1b = np.asarray(w["fc1_b"], N) + np.asarray(w["fc1_w"], N) @ n2b

    f2w = np.asarray(w["fc2_w"], N)
    f2 = np.zeros((128, HS, T, 128), N)
    for hs in range(HS):
        for t in range(T):
            f2[:, hs, t, :] = f2w[t * 128:(t + 1) * 128,
                                  hs * 128:(hs + 1) * 128].T
    out["w_f2"] = bfar(f2.reshape(128, HS * T * 128))

    S = np.zeros((C, C), N)
    ca = np.asarray(w["ca_w"], N)
    for i in range(C):
        for d in range(3):
            j = i + d - 1
            if 0 <= j < C:
                S[i, j] += ca[d]
    fcs = (np.asarray(w["fc_w"], N) + S) / float(L)
    fl = np.zeros((128, T, T, 128), N)
    for kt in range(T):
        for t in range(T):
            fl[:, kt, t, :] = fcs[t * 128:(t + 1) * 128,
                                  kt * 128:(kt + 1) * 128].T
    out["w_fcs"] = bfar(fl.reshape(128, T * T * 128))

    cols = {}
    for g in range(G):
        cols[f"A{g}"] = -np.exp(np.asarray(w["A_log"], N)[g][:, 0])
        cols[f"dtb{g}"] = np.asarray(w["dt_proj_b"], N)[g]
        cols[f"cvb{g}"] = np.asarray(w["conv_b"], N)[g]
        cols[f"Dp{g}"] = np.asarray(w["Dp"], N)[g]
        cols[f"onb{g}"] = np.asarray(w["out_norm_b"], N)[g]
    for t in range(T):
        cols[f"n1b{t}"] = n1b[t * 128:(t + 1) * 128]
        cols[f"pjb{t}"] = pjb[t * 128:(t + 1) * 128]
        cols[f"f2b{t}"] = np.asarray(w["fc2_b"], N)[t * 128:(t + 1) * 128]
        # halved: gate uses tanh(v/2) so the bias enters pre-scaled by 0.5
        cols[f"fcb{t}"] = 0.5 * np.asarray(w["fc_b"], N)[t * 128:(t + 1) * 128]
    for s in range(HS):
        cols[f"f1b{s}"] = f1b[s * 128:(s + 1) * 128]
        cols[f"dwb{s}"] = np.asarray(w["dw_b"], N)[s * 128:(s + 1) * 128]
    vbm = np.zeros((128, len(VB_NAMES)), N)
    for n, i in VB_IDX.items():
        c = cols[n]
        vbm[:, i] = np.concatenate([c, c]) if c.shape[0] == 64 else c
    out["vb"] = vbm
    return out


def input_specs():
    """shapes/dtypes of the device inputs (excluding xt)."""
    import ml_dtypes
    bf = ml_dtypes.bfloat16
    N = np.float32
    return {
        "xt": ((B, T, 128, L), N),
        "w_stF_f": ((128, 4), N),
        "w_stF_h": ((128, 4), bf),
        "w_stG": ((128, 2), bf),
        "w_bc1w": ((2, B * T * 128), bf),
        "w_bci": ((2, B * 128), bf),
        "w_on": ((2, G * 128), bf),
        "w_ipx": ((128, G * 128), bf),
        "w_ipz": ((128, G * 128), bf),
        "w_cv": ((128, G * 9 * 128), bf),
        "w_cvh": ((128, HS * 9 * 128), bf),
        "w_dtx": ((128, G * 128), bf),
        "w_bb": ((128, G * 128), bf),
        "w_cc": ((128, G * 128), bf),
        "w_op": ((128, G * 64), bf),
        "w_pj": ((128, T * T * 128), bf),
        "w_f1": ((128, T * HS * 128), bf),
        "w_f2": ((128, HS * T * 128), bf),
        "w_fcs": ((128, T * T * 128), bf),
        "vb": ((128, len(VB_NAMES)), N),
    }


# ------------------------------------------------------------- device body
def body(ctx: ExitStack, tc, outs, ins):
    nc = tc.nc
    wb = ctx.enter_context(tc.tile_pool(name="wb", bufs=1))
    big = ctx.enter_context(tc.tile_pool(name="big", bufs=1))
    grp = ctx.enter_context(tc.tile_pool(name="grp", bufs=1))
    sc = ctx.enter_context(tc.tile_pool(name="sc", bufs=2))
    # four independent 2-bank PSUM rings so concurrent streams don't
    # serialize through a shared rotation
    ps = ctx.enter_context(tc.tile_pool(name="ps", bufs=2, space="PSUM"))
    psi = ctx.enter_context(tc.tile_pool(name="psi", bufs=2, space="PSUM"))
    psf = ctx.enter_context(tc.tile_pool(name="psf", bufs=2, space="PSUM"))
    ps2 = ctx.enter_context(tc.tile_pool(name="ps2", bufs=2, space="PSUM"))

    ispec = input_specs()

    def wtile(name, engine=None):
        shape, dt = ispec[name]
        t = wb.tile(list(shape), BF16 if dt != np.float32 else F32,
                    tag=name, name=name)
        (engine or nc.sync).dma_start(t, ins[name])
        return t

    # xt first: LN1 stats are the kernel's entry dependency
    xt = [[big.tile([128, L], F32, tag=f"bigf{b * T + t}",
                    name=f"bigf{b * T + t}") for t in range(T)]
          for b in range(B)]
    for b in range(B):
        for t in range(T):
            nc.sync.dma_start(xt[b][t], ins["xt"][b, t])

    w_stF_h = wtile("w_stF_h")
    w_bc1w = wtile("w_bc1w")
    vb = wtile("vb")
    w_ipx = wtile("w_ipx")
    w_ipz = wtile("w_ipz")
    w_stG = wtile("w_stG")
    w_bci = wtile("w_bci")
    w_on = wtile("w_on")
    w_dtx = wtile("w_dtx")
    w_bb = wtile("w_bb")
    w_cc = wtile("w_cc")
    w_op = wtile("w_op")
    w_fcs = wtile("w_fcs")
    w_pj = wtile("w_pj", nc.gpsimd)
    w_f2 = wtile("w_f2", nc.gpsimd)

    def V(name):
        i = VB_IDX[name]
        return vb[:, i:i + 1]

    epsv = wb.tile([128, 1], F32, tag="epsv", name="epsv")
    nc.vector.memset(epsv, EPS)

    # main chunking: 448 cols (8 pixel rows), one PSUM bank per tile
    MCK = 448
    NM = L // MCK            # 7

    def mcs(ck):
        return slice(ck * MCK, (ck + 1) * MCK)

    def cs(ck):
        return slice(ck * CK, (ck + 1) * CK)

    def ppt(parts=128, pool=None):
        return (pool or ps).tile([parts, MCK], F32, tag="pp", name="pp")

    def hv(ap):
        return ap

    def mmsplit(out_ps, lhsT, rhs, start=True, stop=True,
                tile_position=None):
        nc.tensor.matmul(out_ps, lhsT, rhs, start=start, stop=stop,
                         tile_position=tile_position)

    xn = [[big.tile([128, L], BF16, tag=f"xn{b * T + t}",
                    name=f"xn{b * T + t}") for t in range(T)]
          for b in range(B)]

    # ---- stats finisher: arena [:,0]=m -> m*rstd ; [:,1]=E[x^2] -> rstd
    # chunked so downstream apply-matmuls start before the whole row is done
    # same-function chunks batched into blocks: every Ln<->Exp alternation
    # costs a 1.28us ACT_TABLE_LOAD (walrus loads single-anchor sets)
    SF = 784
    def stats_finish(ar):
        m = ar[:, 0]
        q = ar[:, 1]
        for ck in range(L // SF):
            s = slice(ck * SF, (ck + 1) * SF)
            t = sc.tile([2, SF], F32, tag="cf", bufs=2)
            nc.vector.scalar_tensor_tensor(t, m[:, s], -1.0, m[:, s],
                                           OP.mult, OP.mult)
            nc.vector.tensor_add(q[:, s], t, q[:, s])
            nc.scalar.activation(q[:, s], q[:, s], AF.Ln, bias=epsv[0:2])
        for ck in range(L // SF):
            s = slice(ck * SF, (ck + 1) * SF)
            nc.scalar.activation(q[:, s], q[:, s], AF.Exp, scale=-0.5)
            nc.vector.tensor_mul(m[:, s], m[:, s], q[:, s])
        return q, m

    # ---- LN stats helper -> (rstd, m*rstd) [2, L]; 448 chunks (1-bank pp2)
    def ln_stats(tiles, is_f32):
        ar = big.tile([2, 2, L], BF16, tag="st_ar", name="st_ar")
        for ck in range(NCK):
            mps = ps2.tile([2, CK], F32, tag="pp2")
            sps = ps2.tile([2, CK], F32, tag="pp2")
            n = len(tiles)
            for i, (tl, b) in enumerate(tiles):
                lw = w_stF_h[:, 2 * b:2 * b + 2]
                rr = tl[:, cs(ck)]
                if is_f32:
                    xb = sc.tile([128, CK], BF16, tag="cs1", bufs=3)
                    # split casts ACT/DVE: gpsimd is 2.7x slower per op and
                    # serializes the stats matmuls behind its FIFO
                    if i % 2 == 0:
                        nc.vector.tensor_copy(xb, rr)
                    else:
                        nc.scalar.copy(xb, rr)
                    rr = xb
                sq = sc.tile([128, CK], BF16, tag="cs1", bufs=3)
                nc.scalar.activation(sq, tl[:, cs(ck)], AF.Square)
                nc.tensor.matmul(mps, lw, rr, start=(i == 0), stop=(i == n - 1))
                nc.tensor.matmul(sps, lw, sq, start=(i == 0), stop=(i == n - 1))
            nc.vector.tensor_copy(ar[:, 0, cs(ck)], mps)
            nc.scalar.copy(ar[:, 1, cs(ck)], sps)
        return stats_finish(ar)

    def ln_apply(rstd, mr, pairs, lw, bvec=None):
        """each (src, dst): dst = (src - m)*rstd [+b via bvec]; one shared
        broadcast pair per chunk"""
        for ck in range(NM):
            rw = ppt()
            mmsplit(rw, lw, rstd[:, mcs(ck)])
            mw = ppt()
            mmsplit(mw, lw, mr[:, mcs(ck)])
            for src, dst in pairs:
                t1 = hv(sc.tile([128, MCK], F32, tag="c1", name="c1", bufs=3))
                nc.vector.tensor_mul(t1, hv(src[:, mcs(ck)]), rw)
                if bvec is not None:
                    nc.vector.scalar_tensor_tensor(hv(dst[:, mcs(ck)]), t1,
                                                   bvec, mw, OP.add, OP.subtract)
                else:
                    nc.vector.tensor_sub(hv(dst[:, mcs(ck)]), t1, mw)

    # ======== LN1(x) -> xn ========
    r1, m1 = ln_stats([(xt[b][t], b) for b in range(B) for t in range(T)], True)
    for b in range(B):
        for t in range(T):
            lw = w_bc1w[:, (b * T + t) * 128:(b * T + t + 1) * 128]
            ln_apply(r1, m1, [(xt[b][t], xn[b][t])], lw, V(f"n1b{t}"))

    # ======== gate ========
    zs = [[sc.tile([128, 1], BF16, tag=f"zs{b * T + t}", bufs=1,
                   name=f"zs{b * T + t}") for t in range(T)] for b in range(B)]
    gate = [[sc.tile([128, 1], F32, tag=f"gate{b * T + t}", bufs=1,
                     name=f"gate{b * T + t}") for t in range(T)] for b in range(B)]
    for b in range(B):
        for t in range(T):
            with nc.allow_low_precision("bf16 z-sum feeds sigmoid gate"):
                nc.vector.tensor_reduce(zs[b][t], xn[b][t], axis=AX.X, op=OP.add)
    for b in range(B):
        for t in range(T):
            gp = ps2.tile([128, 1], F32, tag="pp2")
            for kt in range(T):
                lw = w_fcs[:, (kt * T + t) * 128:(kt * T + t + 1) * 128]
                nc.tensor.matmul(gp, lw, zs[b][kt],
                                 start=(kt == 0), stop=(kt == T - 1))
            # gate' = 1 + tanh(v/2) = 2*sigmoid(v); the 0.5 is folded into w_op
            nc.scalar.activation(gate[b][t], gp, AF.Tanh,
                                 bias=V(f"fcb{t}"), scale=0.5)
            nc.vector.tensor_scalar_add(gate[b][t], gate[b][t], 1.0)

    # ======== ss2d groups -> ym (pair-interleaved, 784 chunks) ========
    ym = [[big.tile([128, L], BF16, tag=f"bigG{b * T + t}",
                    name=f"bigG{b * T + t}") for t in range(T)]
          for b in range(B)]

    def so_ap(tl, ck, colmajor):
        if not colmajor:
            return tl[:, ck * MCK:(ck + 1) * MCK]
        return bass.AP(tensor=tl.tensor, offset=tl.offset + 8 * ck,
                       ap=[tl.ap[0], [1, 8], [56, 56]])

    padz = [grp.tile([128, LP], BF16, tag=f"padb{j}", name=f"padb{j}")
            for j in range(2)]
    for p_ in padz:
        nc.vector.memset(p_, 0.0)

    ABM_SLOTS = {0: ("bigf1", "bigf2"), 1: ("bigf3", "bigf0")}
    U, SZ, DT, XD, AT, BM, Y = {}, {}, {}, {}, {}, {}, {}
    CVD, STT = {}, {}

    def conv_stage(g):
        colm = g >= 2
        R0 = (g % 2) * 64
        padt = padz[g % 2]
        CVD[g] = grp.tile([128, 9 * 128], BF16, tag=f"cvd{g % 2}",
                          name=f"cvd{g % 2}", bufs=1)
        nc.sync.dma_start(CVD[g], ins["w_cv"][:, g * 9 * 128:(g + 1) * 9 * 128])
        U[g] = grp.tile([128, L], BF16, tag="ub", bufs=2, name=f"u{g}")
        SZ[g] = grp.tile([128, L], BF16, tag="szb", bufs=2, name=f"sz{g}")
        for ck in range(NM):
            xcp = ppt(pool=psi)
            zp = ppt(pool=psi)
            for b in range(B):
                lx = w_ipx[R0:R0 + 64, g * 128 + b * 64:g * 128 + (b + 1) * 64]
                lz = w_ipz[R0:R0 + 64, g * 128 + b * 64:g * 128 + (b + 1) * 64]
                rr = xn[b][g // 2][R0:R0 + 64, mcs(ck)]
                nc.tensor.matmul(xcp[b * 64:(b + 1) * 64], lx, rr,
                                 start=True, stop=True,
                                 tile_position=(R0, b * 64))
                nc.tensor.matmul(zp[b * 64:(b + 1) * 64], lz, rr,
                                 start=True, stop=True,
                                 tile_position=(R0, b * 64))
            dst = bass.AP(tensor=padt.tensor,
                          offset=padt.offset + (1 + 8 * ck) * PW + 1,
                          ap=[padt.ap[0], [PW, 8], [1, 56]])
            nc.vector.tensor_copy(dst, xcp)
            nc.scalar.copy(so_ap(SZ[g], ck, colm), zp)
        for ck in range(NM):
            cvp = ppt(pool=psf)
            for k in range(9):
                dy, dx = k // 3, k % 3
                lhs = CVD[g][:, k * 128:(k + 1) * 128]
                rhs_ = bass.AP(
                    tensor=padt.tensor,
                    offset=padt.offset + (8 * ck + dy) * PW + dx,
                    ap=[padt.ap[0], [PW, 8], [1, 56]])
                nc.tensor.matmul(cvp, lhs, rhs_,
                                 start=(k == 0), stop=(k == 8))
            nc.scalar.copy(so_ap(U[g], ck, colm), cvp)

    def comp_stage(g):
        # silus here (not in conv_stage) so the ACT table set alternates
        # exactly once per group: [silu] then [exp/ln] blocks
        nc.scalar.activation(SZ[g], SZ[g], AF.Silu)
        nc.scalar.activation(U[g], U[g], AF.Silu, bias=V(f"cvb{g}"))
        DT[g] = grp.tile([128, L], BF16, tag="dtb", bufs=2, name=f"dt{g}")
        e1f = grp.tile([128, L], BF16, tag="e1f", bufs=1, name=f"e1f{g}")
        for ck in range(NM):
            dtp = ppt()
            mmsplit(dtp, w_dtx[:, g * 128:(g + 1) * 128], U[g][:, mcs(ck)])
            nc.scalar.activation(e1f[:, mcs(ck)], dtp, AF.Exp, bias=V(f"dtb{g}"))
        for ck in range(NM):
            nc.scalar.activation(DT[g][:, mcs(ck)], e1f[:, mcs(ck)],
                                 AF.Ln, bias=1.0)
        sa, sb = ABM_SLOTS[g % 2]
        AT[g] = big.tile([128, L], BF16, tag=sa, name=f"a{g}")
        BM[g] = big.tile([128, L], BF16, tag=sb, name=f"bm{g}")
        nc.scalar.activation(AT[g], DT[g], AF.Exp, scale=V(f"A{g}"))
        for ck in range(NM):
            bsp = ppt()
            mmsplit(bsp, w_bb[:, g * 128:(g + 1) * 128], U[g][:, mcs(ck)])
            t1 = hv(sc.tile([128, MCK], F32, tag="c1", name="c1", bufs=3))
            nc.vector.tensor_mul(t1, hv(DT[g][:, mcs(ck)]), bsp)
            nc.gpsimd.tensor_mul(hv(BM[g][:, mcs(ck)]), t1,
                                 hv(U[g][:, mcs(ck)]))

    def scan_stage(g):
        if (g % 2) == 1:
            nc.vector.tensor_tensor_scan(BM[g][:, ::-1], AT[g][:, ::-1],
                                         BM[g][:, ::-1], 0.0, OP.mult, OP.add)
        else:
            nc.vector.tensor_tensor_scan(BM[g], AT[g], BM[g],
                                         0.0, OP.mult, OP.add)

    def post_stage(g):
        colm = g >= 2
        R0 = (g % 2) * 64
        Y[g] = grp.tile([128, L], BF16, tag="dtb", bufs=2, name=f"y{g}")
        for ck in range(NM):
            csp = ppt()
            mmsplit(csp, w_cc[:, g * 128:(g + 1) * 128], U[g][:, mcs(ck)])
            t1 = hv(sc.tile([128, MCK], F32, tag="c1", name="c1", bufs=3))
            nc.vector.tensor_mul(t1, hv(BM[g][:, mcs(ck)]), csp)
            nc.vector.scalar_tensor_tensor(hv(Y[g][:, mcs(ck)]),
                                           hv(U[g][:, mcs(ck)]),
                                           V(f"Dp{g}"), t1, OP.mult, OP.add)
        arn = big.tile([2, 2, L], BF16,
                       tag=("st_ar" if g == 0 else
                            "bigf1" if g == 2 else "bigf3"),
                       name=f"st_g{g}")
        for ck in range(NCK):
            ysq = sc.tile([128, CK], BF16, tag="cs1", bufs=3)
            nc.scalar.activation(ysq, Y[g][:, cs(ck)], AF.Square)
            mps = ps2.tile([2, CK], F32, tag="pp2")
            sps = ps2.tile([2, CK], F32, tag="pp2")
            nc.tensor.matmul(mps, w_stG, Y[g][:, cs(ck)], start=True, stop=True)
            nc.tensor.matmul(sps, w_stG, ysq, start=True, stop=True)
            nc.vector.tensor_copy(arn[:, 0, cs(ck)], mps)
            nc.scalar.copy(arn[:, 1, cs(ck)], sps)
        rstd, mr = stats_finish(arn)
        lw_on = w_on[:, g * 128:(g + 1) * 128]
        for ck in range(NM):
            rw = ppt(pool=psi)
            mmsplit(rw, lw_on, rstd[:, mcs(ck)])
            mw = ppt(pool=psi)
            mmsplit(mw, lw_on, mr[:, mcs(ck)])
            t1 = hv(sc.tile([128, MCK], F32, tag="c1", name="c1", bufs=3))
            nc.vector.tensor_mul(t1, hv(Y[g][:, mcs(ck)]), rw)
            nc.vector.scalar_tensor_tensor(t1, t1, V(f"onb{g}"), mw,
                                           OP.add, OP.subtract)
            yh = sc.tile([128, MCK], BF16, tag="c3", bufs=2)
            nc.gpsimd.tensor_mul(hv(yh), t1, hv(SZ[g][:, mcs(ck)]))
            for b in range(B):
                op_ps = ppt()
                lhs = w_op[b * 64:(b + 1) * 64, g * 64:(g + 1) * 64]
                nc.tensor.matmul(op_ps[R0:R0 + 64], lhs,
                                 yh[b * 64:(b + 1) * 64],
                                 start=True, stop=True,
                                 tile_position=(b * 64, R0))
                ymt = ym[b][g // 2]
                xnt = xn[b][g // 2]
                if colm:
                    dst = bass.AP(tensor=ymt.tensor,
                                  offset=ymt.offset + 8 * ck,
                                  ap=[[ymt.ap[0][0], 128], [1, 8],
                                      [56, 56]])[R0:R0 + 64]
                    xnsrc = bass.AP(tensor=xnt.tensor,
                                    offset=xnt.offset + 8 * ck,
                                    ap=[[xnt.ap[0][0], 128], [1, 8],
                                        [56, 56]])[R0:R0 + 64]
                else:
                    dst = hv(ymt[R0:R0 + 64, mcs(ck)])
                    xnsrc = hv(xnt[R0:R0 + 64, mcs(ck)])
                nc.vector.scalar_tensor_tensor(
                    dst, op_ps[R0:R0 + 64], gate[b][g // 2][R0:R0 + 64],
                    xnsrc, OP.mult, OP.mult)

    # LNym stats split into two passes: the t-column finished by groups 0/1
    # is reduced early (its matmuls fill the scan(2)/scan(3) PE stalls); the
    # t=1 pass accumulates into the same arena after group 3
    ymar_h = {}

    def ym_stats_pass(tcol, first):
        if first:
            # allocated lazily: must enter the st_ar ring AFTER arn(0)
            ymar_h["t"] = big.tile([2, 2, L], BF16, tag="st_ar", name="ymar")
        ymar = ymar_h["t"]
        for ck in range(NCK):
            mps = ps2.tile([2, CK], F32, tag="pp2")
            sps = ps2.tile([2, CK], F32, tag="pp2")
            for i in range(B):
                tl = ym[i][tcol]
                lw = w_stF_h[:, 2 * i:2 * i + 2]
                sq = sc.tile([128, CK], BF16, tag="cs1", bufs=3)
                nc.scalar.activation(sq, tl[:, cs(ck)], AF.Square)
                nc.tensor.matmul(mps, lw, tl[:, cs(ck)],
                                 start=(i == 0), stop=(i == B - 1))
                nc.tensor.matmul(sps, lw, sq,
                                 start=(i == 0), stop=(i == B - 1))
            if first:
                nc.vector.tensor_copy(ymar[:, 0, cs(ck)], mps)
                nc.scalar.copy(ymar[:, 1, cs(ck)], sps)
            else:
                nc.vector.tensor_add(ymar[:, 0, cs(ck)],
                                     ymar[:, 0, cs(ck)], mps)
                nc.vector.tensor_add(ymar[:, 1, cs(ck)],
                                     ymar[:, 1, cs(ck)], sps)

    # software-pipelined group schedule: conv work of group g+1 is emitted
    # before the scan of group g so PE (and the DVE FIFO) have dense work
    # while the 6.7us scan runs
    conv_stage(0)
    for g in range(G):
        comp_stage(g)
        if g + 1 < G:
            conv_stage(g + 1)
        scan_stage(g)
        post_stage(g)
        if g == 1:
            ym_stats_pass(0, True)

    # ======== LN1(ym) in-place -> ymhat; proj; x2 = xt + proj + b ========
    # reload x in slot-death order (bigf1 frees at scan(2), bigf2 at post(2),
    # bigf3 at scan(3), bigf0 at post(3)) so the sync DMA FIFO never
    # head-of-line blocks on the last-freed slot
    for b, t in ((0, 1), (1, 0), (1, 1), (0, 0)):
        xt[b][t] = big.tile([128, L], F32, tag=f"bigf{b * T + t}",
                            name=f"xt2_{b * T + t}")
        nc.sync.dma_start(xt[b][t], ins["xt"][b, t])
    ym_stats_pass(1, False)
    rym, mym = stats_finish(ymar_h["t"])
    for b in range(B):
        lw = w_bci[:, b * 128:(b + 1) * 128]
        ln_apply(rym, mym, [(ym[b][t], ym[b][t]) for t in range(T)], lw)
    for b in range(B):
        for t in range(T):
            for ck in range(NM):
                # psi ring (idle here): keeps proj off the pp ring so it
                # pipelines per-chunk with the LNym apply instead of
                # queueing behind all of its slot allocations
                xp = ppt(pool=psi)
                for kt in range(T):
                    lhs = w_pj[:, (t * T + kt) * 128:(t * T + kt + 1) * 128]
                    mmsplit(xp, lhs, ym[b][kt][:, mcs(ck)],
                            start=(kt == 0), stop=(kt == T - 1))
                nc.vector.scalar_tensor_tensor(
                    hv(xt[b][t][:, mcs(ck)]), xp, V(f"pjb{t}"),
                    hv(xt[b][t][:, mcs(ck)]), OP.add, OP.add)

    # ======== LN2 -> xhat2 (xn slots); spill x2 to DRAM ========
    r2, m2 = ln_stats([(xt[b][t], b) for b in range(B) for t in range(T)], True)
    xh2 = [[big.tile([128, L], BF16, tag=f"xn{b * T + t}",
                     name=f"xh2_{b * T + t}") for t in range(T)]
           for b in range(B)]
    for b in range(B):
        lw = w_bci[:, b * 128:(b + 1) * 128]
        ln_apply(r2, m2, [(xt[b][t], xh2[b][t]) for t in range(T)], lw)

    # ======== MLP ======== (x2 stays resident in the bigf slots; gel lives
    # in the dead ym/U/SZ slots, so no DRAM spill round-trip is needed)
    for b in range(B):
        gels = ([big.tile([128, L], BF16, tag=f"bigG{j}", name=f"gel{b}_{j}")
                 for j in range(4)]
                + [grp.tile([128, L], BF16, tag="ub", bufs=2,
                            name=f"gelu{b}_{j}") for j in range(2)]
                + [grp.tile([128, L], BF16, tag="szb", bufs=2,
                            name=f"gelz{b}_{j}") for j in range(2)])
        for s in range(HS):
            f1s = grp.tile([128, 2 * 128], BF16, tag=f"f1s{s % 2}",
                           name=f"f1s{s % 2}", bufs=1)
            nc.gpsimd.dma_start(
                f1s[:, 0:128], ins["w_f1"][:, (0 * HS + s) * 128:(0 * HS + s + 1) * 128])
            nc.gpsimd.dma_start(
                f1s[:, 128:256], ins["w_f1"][:, (1 * HS + s) * 128:(1 * HS + s + 1) * 128])
            cvhd = grp.tile([128, 9 * 128], BF16, tag=f"cvd{s % 2}",
                            name=f"cvhd{s % 2}", bufs=1)
            nc.gpsimd.dma_start(cvhd,
                                ins["w_cvh"][:, s * 9 * 128:(s + 1) * 9 * 128])
            padt = padz[s % 2]
            for ck in range(NM):
                hp = ppt()
                for kt in range(T):
                    lhs = f1s[:, kt * 128:(kt + 1) * 128]
                    mmsplit(hp, lhs, xh2[b][kt][:, mcs(ck)],
                            start=(kt == 0), stop=(kt == T - 1))
                dst = bass.AP(tensor=padt.tensor,
                              offset=padt.offset + (1 + 8 * ck) * PW + 1,
                              ap=[padt.ap[0], [PW, 8], [1, 56]])
                nc.vector.tensor_copy(dst, hp)
            for ck in range(NM):
                cvp = ppt(pool=psf if ck % 2 == 0 else psi)
                for k in range(9):
                    dy, dx = k // 3, k % 3
                    lhs = cvhd[:, k * 128:(k + 1) * 128]
                    rhs_ = bass.AP(
                        tensor=padt.tensor,
                        offset=padt.offset + (8 * ck + dy) * PW + dx,
                        ap=[padt.ap[0], [PW, 8], [1, 56]])
                    nc.tensor.matmul(cvp, lhs, rhs_,
                                     start=(k == 0), stop=(k == 8))
                nc.scalar.activation(hv(gels[s][:, mcs(ck)]), cvp,
                                     AF.Gelu, bias=V(f"dwb{s}"))
        for t in range(T):
            for ck in range(NM):
                xp = ppt()
                for s in range(HS):
                    lhs = w_f2[:, (s * T + t) * 128:(s * T + t + 1) * 128]
                    mmsplit(xp, lhs, gels[s][:, mcs(ck)],
                            start=(s == 0), stop=(s == HS - 1))
                x3 = sc.tile([128, MCK], F32, tag="c1", bufs=3)
                nc.vector.scalar_tensor_tensor(
                    hv(x3), xp, V(f"f2b{t}"), hv(xt[b][t][:, mcs(ck)]),
                    OP.add, OP.add)
                nc.sync.dma_start(outs["out"][b, t, :, mcs(ck)], x3)


# =================================================================
# Runner: full-input kernel() entry point. Shards batch over 8 cores,
# compiles the Bass module once, runs SPMD via PJRT/axon, gathers.
# =================================================================
import concourse.bacc as bacc
import concourse.bass_utils as bass_utils

N_CORES = 8
_CACHE = {}


def _build_nc():
    if "nc" in _CACHE:
        return _CACHE["nc"]
    nc = bacc.Bacc("TRN2", debug=False, num_devices=N_CORES)
    ispec = input_specs()
    ins = {}
    for name, (shape, dt) in ispec.items():
        mdt = F32 if dt == np.float32 else BF16
        ins[name] = nc.dram_tensor(name, shape, mdt, kind="ExternalInput").ap()
    out = nc.dram_tensor("out", (B, T, 128, L), F32, kind="ExternalOutput").ap()
    import concourse.tile as _tile
    with _tile.TileContext(nc) as tc:
        with ExitStack() as ctx:
            body(ctx, tc, {"out": out}, ins)
    nc.compile()
    _CACHE["nc"] = nc
    return nc


def make_in_maps(inputs):
    x = np.asarray(inputs["x"], dtype=np.float32)
    return [host_prep(x[c * B:(c + 1) * B], inputs) for c in range(N_CORES)]


def kernel(**inputs):
    nc = _build_nc()
    in_maps = make_in_maps(inputs)
    res = bass_utils.run_bass_kernel_spmd(nc, in_maps,
                                          core_ids=list(range(N_CORES)))
    outs = []
    for c in range(N_CORES):
        o = res.results[c]["out"].reshape(B, C, L).transpose(0, 2, 1)
        outs.append(o)
    return np.ascontiguousarray(np.concatenate(outs, axis=0))



# revision 42
# speedup vs baseline: 1.0149x; 1.0008x over previous
"""GroupMamba block kernel for TRN2 — per-core body + host weight prep.

Per-core work: 2 batches of the (16, 3136, 256) problem. Layout is
channel-partition: activations live as [128 ch, L=3136] tiles, one per
(batch, ctile). All cross-partition ops (LN stats, broadcasts, dwconv,
projections) go through the TensorEngine with host-built block matrices.
The Mamba recurrence is a single tensor_tensor_scan per group.
"""
import numpy as np
from contextlib import ExitStack

import concourse.bass as bass
import concourse.tile as tile  # noqa: F401
from concourse import mybir

F32 = mybir.dt.float32
F32R = mybir.dt.float32r
BF16 = mybir.dt.bfloat16
AF = mybir.ActivationFunctionType
OP = mybir.AluOpType
AX = mybir.AxisListType

B = 2          # batches per core
T = 2          # ctiles (256 = 2*128)
G = 4          # ss2d groups
Cg = 64
C = 256
H = W = 56
L = H * W      # 3136
HID = 1024
HS = 8         # hidden slices of 128
CK = 448       # L-chunk (8 pixel rows)
NCK = L // CK  # 7
PW = 64        # padded row stride
PR = 58        # padded rows
LP = PR * PW   # 3712
EPS = 1e-5

VB_NAMES = ([f"A{g}" for g in range(G)] + [f"dtb{g}" for g in range(G)]
            + [f"cvb{g}" for g in range(G)] + [f"Dp{g}" for g in range(G)]
            + [f"onb{g}" for g in range(G)] + [f"n1b{t}" for t in range(T)]
            + [f"pjb{t}" for t in range(T)] + [f"f1b{s}" for s in range(HS)]
            + [f"f2b{t}" for t in range(T)] + [f"dwb{s}" for s in range(HS)]
            + [f"fcb{t}" for t in range(T)])
VB_IDX = {n: i for i, n in enumerate(VB_NAMES)}


def f32r(ap):
    return ap.bitcast(F32R)


# ---------------------------------------------------------------- host prep
def host_prep(x2b, w):
    """x2b: (2, 3136, 256) f32 shard; w: dict of full weights.
    Returns the per-core device input map (numpy arrays)."""
    import ml_dtypes
    bf = ml_dtypes.bfloat16
    N = np.float32

    def bfar(a):
        return np.ascontiguousarray(np.asarray(a, dtype=np.float32)).astype(bf)

    out = {}
    xt = np.asarray(x2b, dtype=N).transpose(0, 2, 1).reshape(B, T, 128, L)
    out["xt"] = np.ascontiguousarray(xt)

    n1w = np.asarray(w["norm1_w"], N); n1b = np.asarray(w["norm1_b"], N)
    n2w = np.asarray(w["norm2_w"], N); n2b = np.asarray(w["norm2_b"], N)
    skip = float(np.asarray(w["skip_scale"]).reshape(-1)[0])

    stF = np.zeros((128, 4), N)
    stF[:, 0] = 1.0 / C
    stF[:, 3] = 1.0 / C
    out["w_stF_f"] = stF
    out["w_stF_h"] = bfar(stF)
    stG = np.zeros((128, 2), N)
    stG[:64, 0] = 1.0 / Cg
    stG[64:, 1] = 1.0 / Cg
    out["w_stG"] = bfar(stG)

    bc1w = np.zeros((2, B * T * 128), N)
    for b in range(B):
        for t in range(T):
            bc1w[b, (b * T + t) * 128:(b * T + t + 1) * 128] = \
                n1w[t * 128:(t + 1) * 128]
    out["w_bc1w"] = bfar(bc1w)
    bci = np.zeros((2, B * 128), N)
    bci[0, :128] = 1.0
    bci[1, 128:] = 1.0
    out["w_bci"] = bfar(bci)
    bon = np.zeros((2, G * 128), N)
    onw = np.asarray(w["out_norm_w"], N)
    for g in range(G):
        bon[0, g * 128:g * 128 + 64] = onw[g]
        bon[1, g * 128 + 64:(g + 1) * 128] = onw[g]
    out["w_on"] = bfar(bon)

    ipw = np.asarray(w["in_proj_w"], N)
    ipx = np.zeros((128, G, 128), N)
    ipz = np.zeros((128, G, 128), N)
    for g in range(G):
        R = (g % 2) * 64
        bx = ipw[g][:64].T
        bz = ipw[g][64:].T
        ipx[R:R + 64, g, 0:64] = bx
        ipx[R:R + 64, g, 64:128] = bx
        ipz[R:R + 64, g, 0:64] = bz
        ipz[R:R + 64, g, 64:128] = bz
    out["w_ipx"] = bfar(ipx.reshape(128, G * 128))
    out["w_ipz"] = bfar(ipz.reshape(128, G * 128))

    cw = np.asarray(w["conv_w"], N)
    cvd = np.zeros((128, G, 9, 128), N)
    for g in range(G):
        for k in range(9):
            v = cw[g, :, k // 3, k % 3]
            cvd[np.arange(128), g, k, np.arange(128)] = np.concatenate([v, v])
    out["w_cv"] = bfar(cvd.reshape(128, G * 9 * 128))

    dww = np.asarray(w["dw_w"], N)
    cvh = np.zeros((128, HS, 9, 128), N)
    for s in range(HS):
        for k in range(9):
            v = dww[s * 128:(s + 1) * 128, k // 3, k % 3]
            cvh[np.arange(128), s, k, np.arange(128)] = v
    out["w_cvh"] = bfar(cvh.reshape(128, HS * 9 * 128))

    # x_proj and dt_proj folded into single per-group [64,64] / rank-1
    # matrices applied directly to u (no XD intermediate on device)
    xpw = np.asarray(w["x_proj_w"], N)
    dtw = np.asarray(w["dt_proj_w"], N)
    dtx = np.zeros((128, G, 128), N)
    bbl = np.zeros((128, G, 128), N)
    ccl = np.zeros((128, G, 128), N)
    for g in range(G):
        blk = (dtw[g] @ xpw[g][:4]).T          # [k, c]
        for b in range(2):
            sl = slice(b * 64, (b + 1) * 64)
            dtx[sl, g, sl] = blk
            bbl[sl, g, sl] = xpw[g][4][:, None]
            ccl[sl, g, sl] = xpw[g][5][:, None]
    out["w_dtx"] = bfar(dtx.reshape(128, G * 128))
    out["w_bb"] = bfar(bbl.reshape(128, G * 128))
    out["w_cc"] = bfar(ccl.reshape(128, G * 128))

    opw = np.asarray(w["out_proj_w"], N)
    opl = np.zeros((128, G, 64), N)
    for g in range(G):
        # extra 0.5: gate is computed as 1+tanh(v/2) = 2*sigmoid(v)
        blk = (opw[g] * skip * 0.5).T
        opl[0:64, g] = blk
        opl[64:128, g] = blk
    out["w_op"] = bfar(opl.reshape(128, G * 64))

    pw = np.asarray(w["proj_w"], N) * n1w[None, :]
    pj = np.zeros((128, T, T, 128), N)
    for t in range(T):
        for kt in range(T):
            pj[:, t, kt, :] = pw[t * 128:(t + 1) * 128,
                                 kt * 128:(kt + 1) * 128].T
    out["w_pj"] = bfar(pj.reshape(128, T * T * 128))
    pjb = np.asarray(w["proj_b"], N) + np.asarray(w["proj_w"], N) @ n1b

    f1w = np.asarray(w["fc1_w"], N) * n2w[None, :]
    f1 = np.zeros((128, T, HS, 128), N)
    for kt in range(T):
        for hs in range(HS):
            f1[:, kt, hs, :] = f1w[hs * 128:(hs + 1) * 128,
                                   kt * 128:(kt + 1) * 128].T
    out["w_f1"] = bfar(f1.reshape(128, T * HS * 128))
    f1b = np.asarray(w["fc1_b"], N) + np.asarray(w["fc1_w"], N) @ n2b

    f2w = np.asarray(w["fc2_w"], N)
    f2 = np.zeros((128, HS, T, 128), N)
    for hs in range(HS):
        for t in range(T):
            f2[:, hs, t, :] = f2w[t * 128:(t + 1) * 128,
                                  hs * 128:(hs + 1) * 128].T
    out["w_f2"] = bfar(f2.reshape(128, HS * T * 128))

    S = np.zeros((C, C), N)
    ca = np.asarray(w["ca_w"], N)
    for i in range(C):
        for d in range(3):
            j = i + d - 1
            if 0 <= j < C:
                S[i, j] += ca[d]
    fcs = (np.asarray(w["fc_w"], N) + S) / float(L)
    fl = np.zeros((128, T, T, 128), N)
    for kt in range(T):
        for t in range(T):
            fl[:, kt, t, :] = fcs[t * 128:(t + 1) * 128,
                                  kt * 128:(kt + 1) * 128].T
    out["w_fcs"] = bfar(fl.reshape(128, T * T * 128))

    cols = {}
    for g in range(G):
        cols[f"A{g}"] = -np.exp(np.asarray(w["A_log"], N)[g][:, 0])
        cols[f"dtb{g}"] = np.asarray(w["dt_proj_b"], N)[g]
        cols[f"cvb{g}"] = np.asarray(w["conv_b"], N)[g]
        cols[f"Dp{g}"] = np.asarray(w["Dp"], N)[g]
        cols[f"onb{g}"] = np.asarray(w["out_norm_b"], N)[g]
    for t in range(T):
        cols[f"n1b{t}"] = n1b[t * 128:(t + 1) * 128]
        cols[f"pjb{t}"] = pjb[t * 128:(t + 1) * 128]
        cols[f"f2b{t}"] = np.asarray(w["fc2_b"], N)[t * 128:(t + 1) * 128]
        # halved: gate uses tanh(v/2) so the bias enters pre-scaled by 0.5
        cols[f"fcb{t}"] = 0.5 * np.asarray(w["fc_b"], N)[t * 128:(t + 1) * 128]
    for s in range(HS):
        cols[f"f1b{s}"] = f1b[s * 128:(s + 1) * 128]
        cols[f"dwb{s}"] = np.asarray(w["dw_b"], N)[s * 128:(s + 1) * 128]
    vbm = np.zeros((128, len(VB_NAMES)), N)
    for n, i in VB_IDX.items():
        c = cols[n]
        vbm[:, i] = np.concatenate([c, c]) if c.shape[0] == 64 else c
    out["vb"] = vbm
    return out


def input_specs():
    """shapes/dtypes of the device inputs (excluding xt)."""
    import ml_dtypes
    bf = ml_dtypes.bfloat16
    N = np.float32
    return {
        "xt": ((B, T, 128, L), N),
        "w_stF_f": ((128, 4), N),
        "w_stF_h": ((128, 4), bf),
        "w_stG": ((128, 2), bf),
        "w_bc1w": ((2, B * T * 128), bf),
        "w_bci": ((2, B * 128), bf),
        "w_on": ((2, G * 128), bf),
        "w_ipx": ((128, G * 128), bf),
        "w_ipz": ((128, G * 128), bf),
        "w_cv": ((128, G * 9 * 128), bf),
        "w_cvh": ((128, HS * 9 * 128), bf),
        "w_dtx": ((128, G * 128), bf),
        "w_bb": ((128, G * 128), bf),
        "w_cc": ((128, G * 128), bf),
        "w_op": ((128, G * 64), bf),
        "w_pj": ((128, T * T * 128), bf),
        "w_f1": ((128, T * HS * 128), bf),
        "w_f2": ((128, HS * T * 128), bf),
        "w_fcs": ((128, T * T * 128), bf),
        "vb": ((128, len(VB_NAMES)), N),
    }


# ------------------------------------------------------------- device body
def body(ctx: ExitStack, tc, outs, ins):
    nc = tc.nc
    wb = ctx.enter_context(tc.tile_pool(name="wb", bufs=1))
    big = ctx.enter_context(tc.tile_pool(name="big", bufs=1))
    grp = ctx.enter_context(tc.tile_pool(name="grp", bufs=1))
    sc = ctx.enter_context(tc.tile_pool(name="sc", bufs=2))
    # four independent 2-bank PSUM rings so concurrent streams don't
    # serialize through a shared rotation
    ps = ctx.enter_context(tc.tile_pool(name="ps", bufs=2, space="PSUM"))
    psi = ctx.enter_context(tc.tile_pool(name="psi", bufs=2, space="PSUM"))
    psf = ctx.enter_context(tc.tile_pool(name="psf", bufs=2, space="PSUM"))
    ps2 = ctx.enter_context(tc.tile_pool(name="ps2", bufs=2, space="PSUM"))

    ispec = input_specs()

    def wtile(name, engine=None):
        shape, dt = ispec[name]
        t = wb.tile(list(shape), BF16 if dt != np.float32 else F32,
                    tag=name, name=name)
        (engine or nc.sync).dma_start(t, ins[name])
        return t

    # xt first: LN1 stats are the kernel's entry dependency
    xt = [[big.tile([128, L], F32, tag=f"bigf{b * T + t}",
                    name=f"bigf{b * T + t}") for t in range(T)]
          for b in range(B)]
    for b in range(B):
        for t in range(T):
            nc.sync.dma_start(xt[b][t], ins["xt"][b, t])

    w_stF_h = wtile("w_stF_h")
    w_bc1w = wtile("w_bc1w")
    vb = wtile("vb")
    w_ipx = wtile("w_ipx")
    w_ipz = wtile("w_ipz")
    w_stG = wtile("w_stG")
    w_bci = wtile("w_bci")
    w_on = wtile("w_on")
    w_dtx = wtile("w_dtx")
    w_bb = wtile("w_bb")
    w_cc = wtile("w_cc")
    w_op = wtile("w_op")
    w_fcs = wtile("w_fcs")
    w_pj = wtile("w_pj", nc.gpsimd)
    w_f2 = wtile("w_f2", nc.gpsimd)

    def V(name):
        i = VB_IDX[name]
        return vb[:, i:i + 1]

    epsv = wb.tile([128, 1], F32, tag="epsv", name="epsv")
    nc.vector.memset(epsv, EPS)

    # main chunking: 448 cols (8 pixel rows), one PSUM bank per tile
    MCK = 448
    NM = L // MCK            # 7

    def mcs(ck):
        return slice(ck * MCK, (ck + 1) * MCK)

    def cs(ck):
        return slice(ck * CK, (ck + 1) * CK)

    def ppt(parts=128, pool=None):
        return (pool or ps).tile([parts, MCK], F32, tag="pp", name="pp")

    def hv(ap):
        return ap

    def mmsplit(out_ps, lhsT, rhs, start=True, stop=True,
                tile_position=None):
        nc.tensor.matmul(out_ps, lhsT, rhs, start=start, stop=stop,
                         tile_position=tile_position)

    xn = [[big.tile([128, L], BF16, tag=f"xn{b * T + t}",
                    name=f"xn{b * T + t}") for t in range(T)]
          for b in range(B)]

    # ---- stats finisher: arena [:,0]=m -> m*rstd ; [:,1]=E[x^2] -> rstd
    # chunked so downstream apply-matmuls start before the whole row is done
    # same-function chunks batched into blocks: every Ln<->Exp alternation
    # costs a 1.28us ACT_TABLE_LOAD (walrus loads single-anchor sets)
    SF = 784
    def stats_finish(ar):
        m = ar[:, 0]
        q = ar[:, 1]
        for ck in range(L // SF):
            s = slice(ck * SF, (ck + 1) * SF)
            t = sc.tile([2, SF], F32, tag="cf", bufs=2)
            nc.vector.scalar_tensor_tensor(t, m[:, s], -1.0, m[:, s],
                                           OP.mult, OP.mult)
            nc.vector.tensor_add(q[:, s], t, q[:, s])
            nc.scalar.activation(q[:, s], q[:, s], AF.Ln, bias=epsv[0:2])
        for ck in range(L // SF):
            s = slice(ck * SF, (ck + 1) * SF)
            nc.scalar.activation(q[:, s], q[:, s], AF.Exp, scale=-0.5)
            nc.vector.tensor_mul(m[:, s], m[:, s], q[:, s])
        return q, m

    # ---- LN stats helper -> (rstd, m*rstd) [2, L]; 448 chunks (1-bank pp2)
    def ln_stats(tiles, is_f32):
        ar = big.tile([2, 2, L], BF16, tag="st_ar", name="st_ar")
        for ck in range(NCK):
            mps = ps2.tile([2, CK], F32, tag="pp2")
            sps = ps2.tile([2, CK], F32, tag="pp2")
            n = len(tiles)
            for i, (tl, b) in enumerate(tiles):
                lw = w_stF_h[:, 2 * b:2 * b + 2]
                rr = tl[:, cs(ck)]
                if is_f32:
                    xb = sc.tile([128, CK], BF16, tag="cs1", bufs=3)
                    # split casts ACT/DVE: gpsimd is 2.7x slower per op and
                    # serializes the stats matmuls behind its FIFO
                    if i % 2 == 0:
                        nc.vector.tensor_copy(xb, rr)
                    else:
                        nc.scalar.copy(xb, rr)
                    rr = xb
                sq = sc.tile([128, CK], BF16, tag="cs1", bufs=3)
                nc.scalar.activation(sq, tl[:, cs(ck)], AF.Square)
                nc.tensor.matmul(mps, lw, rr, start=(i == 0), stop=(i == n - 1))
                nc.tensor.matmul(sps, lw, sq, start=(i == 0), stop=(i == n - 1))
            nc.vector.tensor_copy(ar[:, 0, cs(ck)], mps)
            nc.scalar.copy(ar[:, 1, cs(ck)], sps)
        return stats_finish(ar)

    def ln_apply(rstd, mr, pairs, lw, bvec=None):
        """each (src, dst): dst = (src - m)*rstd [+b via bvec]; one shared
        broadcast pair per chunk"""
        for ck in range(NM):
            rw = ppt()
            mmsplit(rw, lw, rstd[:, mcs(ck)])
            mw = ppt()
            mmsplit(mw, lw, mr[:, mcs(ck)])
            for src, dst in pairs:
                t1 = hv(sc.tile([128, MCK], F32, tag="c1", name="c1", bufs=4))
                nc.vector.tensor_mul(t1, hv(src[:, mcs(ck)]), rw)
                if bvec is not None:
                    nc.vector.scalar_tensor_tensor(hv(dst[:, mcs(ck)]), t1,
                                                   bvec, mw, OP.add, OP.subtract)
                else:
                    nc.vector.tensor_sub(hv(dst[:, mcs(ck)]), t1, mw)

    # ======== LN1(x) -> xn ========
    r1, m1 = ln_stats([(xt[b][t], b) for b in range(B) for t in range(T)], True)
    for b in range(B):
        for t in range(T):
            lw = w_bc1w[:, (b * T + t) * 128:(b * T + t + 1) * 128]
            ln_apply(r1, m1, [(xt[b][t], xn[b][t])], lw, V(f"n1b{t}"))

    # ======== gate ========
    zs = [[sc.tile([128, 1], BF16, tag=f"zs{b * T + t}", bufs=1,
                   name=f"zs{b * T + t}") for t in range(T)] for b in range(B)]
    gate = [[sc.tile([128, 1], F32, tag=f"gate{b * T + t}", bufs=1,
                     name=f"gate{b * T + t}") for t in range(T)] for b in range(B)]
    for b in range(B):
        for t in range(T):
            with nc.allow_low_precision("bf16 z-sum feeds sigmoid gate"):
                nc.vector.tensor_reduce(zs[b][t], xn[b][t], axis=AX.X, op=OP.add)
    for b in range(B):
        for t in range(T):
            gp = ps2.tile([128, 1], F32, tag="pp2")
            for kt in range(T):
                lw = w_fcs[:, (kt * T + t) * 128:(kt * T + t + 1) * 128]
                nc.tensor.matmul(gp, lw, zs[b][kt],
                                 start=(kt == 0), stop=(kt == T - 1))
            # gate' = 1 + tanh(v/2) = 2*sigmoid(v); the 0.5 is folded into w_op
            nc.scalar.activation(gate[b][t], gp, AF.Tanh,
                                 bias=V(f"fcb{t}"), scale=0.5)
            nc.vector.tensor_scalar_add(gate[b][t], gate[b][t], 1.0)

    # ======== ss2d groups -> ym (pair-interleaved, 784 chunks) ========
    ym = [[big.tile([128, L], BF16, tag=f"bigG{b * T + t}",
                    name=f"bigG{b * T + t}") for t in range(T)]
          for b in range(B)]

    def so_ap(tl, ck, colmajor):
        if not colmajor:
            return tl[:, ck * MCK:(ck + 1) * MCK]
        return bass.AP(tensor=tl.tensor, offset=tl.offset + 8 * ck,
                       ap=[tl.ap[0], [1, 8], [56, 56]])

    padz = [grp.tile([128, LP], BF16, tag=f"padb{j}", name=f"padb{j}")
            for j in range(2)]
    for p_ in padz:
        nc.vector.memset(p_, 0.0)

    ABM_SLOTS = {0: ("bigf1", "bigf2"), 1: ("bigf3", "bigf0")}
    U, SZ, DT, XD, AT, BM, Y = {}, {}, {}, {}, {}, {}, {}
    CVD, STT = {}, {}

    def conv_stage(g):
        colm = g >= 2
        R0 = (g % 2) * 64
        padt = padz[g % 2]
        CVD[g] = grp.tile([128, 9 * 128], BF16, tag=f"cvd{g % 2}",
                          name=f"cvd{g % 2}", bufs=1)
        nc.sync.dma_start(CVD[g], ins["w_cv"][:, g * 9 * 128:(g + 1) * 9 * 128])
        U[g] = grp.tile([128, L], BF16, tag="ub", bufs=2, name=f"u{g}")
        SZ[g] = grp.tile([128, L], BF16, tag="szb", bufs=2, name=f"sz{g}")
        for ck in range(NM):
            xcp = ppt(pool=psi)
            zp = ppt(pool=psi)
            for b in range(B):
                lx = w_ipx[R0:R0 + 64, g * 128 + b * 64:g * 128 + (b + 1) * 64]
                lz = w_ipz[R0:R0 + 64, g * 128 + b * 64:g * 128 + (b + 1) * 64]
                rr = xn[b][g // 2][R0:R0 + 64, mcs(ck)]
                nc.tensor.matmul(xcp[b * 64:(b + 1) * 64], lx, rr,
                                 start=True, stop=True,
                                 tile_position=(R0, b * 64))
                nc.tensor.matmul(zp[b * 64:(b + 1) * 64], lz, rr,
                                 start=True, stop=True,
                                 tile_position=(R0, b * 64))
            dst = bass.AP(tensor=padt.tensor,
                          offset=padt.offset + (1 + 8 * ck) * PW + 1,
                          ap=[padt.ap[0], [PW, 8], [1, 56]])
            nc.vector.tensor_copy(dst, xcp)
            nc.scalar.copy(so_ap(SZ[g], ck, colm), zp)
        for ck in range(NM):
            cvp = ppt(pool=psf)
            for k in range(9):
                dy, dx = k // 3, k % 3
                lhs = CVD[g][:, k * 128:(k + 1) * 128]
                rhs_ = bass.AP(
                    tensor=padt.tensor,
                    offset=padt.offset + (8 * ck + dy) * PW + dx,
                    ap=[padt.ap[0], [PW, 8], [1, 56]])
                nc.tensor.matmul(cvp, lhs, rhs_,
                                 start=(k == 0), stop=(k == 8))
            nc.scalar.copy(so_ap(U[g], ck, colm), cvp)

    def comp_stage(g):
        # silus here (not in conv_stage) so the ACT table set alternates
        # exactly once per group: [silu] then [exp/ln] blocks
        nc.scalar.activation(SZ[g], SZ[g], AF.Silu)
        nc.scalar.activation(U[g], U[g], AF.Silu, bias=V(f"cvb{g}"))
        DT[g] = grp.tile([128, L], BF16, tag="dtb", bufs=2, name=f"dt{g}")
        e1f = grp.tile([128, L], BF16, tag="e1f", bufs=1, name=f"e1f{g}")
        for ck in range(NM):
            dtp = ppt()
            mmsplit(dtp, w_dtx[:, g * 128:(g + 1) * 128], U[g][:, mcs(ck)])
            nc.scalar.activation(e1f[:, mcs(ck)], dtp, AF.Exp, bias=V(f"dtb{g}"))
        for ck in range(NM):
            nc.scalar.activation(DT[g][:, mcs(ck)], e1f[:, mcs(ck)],
                                 AF.Ln, bias=1.0)
        sa, sb = ABM_SLOTS[g % 2]
        AT[g] = big.tile([128, L], BF16, tag=sa, name=f"a{g}")
        BM[g] = big.tile([128, L], BF16, tag=sb, name=f"bm{g}")
        nc.scalar.activation(AT[g], DT[g], AF.Exp, scale=V(f"A{g}"))
        for ck in range(NM):
            bsp = ppt()
            mmsplit(bsp, w_bb[:, g * 128:(g + 1) * 128], U[g][:, mcs(ck)])
            t1 = hv(sc.tile([128, MCK], F32, tag="c1", name="c1", bufs=4))
            nc.vector.tensor_mul(t1, hv(DT[g][:, mcs(ck)]), bsp)
            nc.gpsimd.tensor_mul(hv(BM[g][:, mcs(ck)]), t1,
                                 hv(U[g][:, mcs(ck)]))

    def scan_stage(g):
        if (g % 2) == 1:
            nc.vector.tensor_tensor_scan(BM[g][:, ::-1], AT[g][:, ::-1],
                                         BM[g][:, ::-1], 0.0, OP.mult, OP.add)
        else:
            nc.vector.tensor_tensor_scan(BM[g], AT[g], BM[g],
                                         0.0, OP.mult, OP.add)

    def post_stage(g):
        colm = g >= 2
        R0 = (g % 2) * 64
        Y[g] = grp.tile([128, L], BF16, tag="dtb", bufs=2, name=f"y{g}")
        for ck in range(NM):
            csp = ppt()
            mmsplit(csp, w_cc[:, g * 128:(g + 1) * 128], U[g][:, mcs(ck)])
            t1 = hv(sc.tile([128, MCK], F32, tag="c1", name="c1", bufs=4))
            nc.vector.tensor_mul(t1, hv(BM[g][:, mcs(ck)]), csp)
            nc.vector.scalar_tensor_tensor(hv(Y[g][:, mcs(ck)]),
                                           hv(U[g][:, mcs(ck)]),
                                           V(f"Dp{g}"), t1, OP.mult, OP.add)
        arn = big.tile([2, 2, L], BF16,
                       tag=("st_ar" if g == 0 else
                            "bigf1" if g == 2 else "bigf3"),
                       name=f"st_g{g}")
        for ck in range(NCK):
            ysq = sc.tile([128, CK], BF16, tag="cs1", bufs=3)
            nc.scalar.activation(ysq, Y[g][:, cs(ck)], AF.Square)
            mps = ps2.tile([2, CK], F32, tag="pp2")
            sps = ps2.tile([2, CK], F32, tag="pp2")
            nc.tensor.matmul(mps, w_stG, Y[g][:, cs(ck)], start=True, stop=True)
            nc.tensor.matmul(sps, w_stG, ysq, start=True, stop=True)
            nc.vector.tensor_copy(arn[:, 0, cs(ck)], mps)
            nc.scalar.copy(arn[:, 1, cs(ck)], sps)
        rstd, mr = stats_finish(arn)
        lw_on = w_on[:, g * 128:(g + 1) * 128]
        for ck in range(NM):
            rw = ppt(pool=psi)
            mmsplit(rw, lw_on, rstd[:, mcs(ck)])
            mw = ppt(pool=psi)
            mmsplit(mw, lw_on, mr[:, mcs(ck)])
            t1 = hv(sc.tile([128, MCK], F32, tag="c1", name="c1", bufs=4))
            nc.vector.tensor_mul(t1, hv(Y[g][:, mcs(ck)]), rw)
            nc.vector.scalar_tensor_tensor(t1, t1, V(f"onb{g}"), mw,
                                           OP.add, OP.subtract)
            yh = sc.tile([128, MCK], BF16, tag="c3", bufs=2)
            nc.gpsimd.tensor_mul(hv(yh), t1, hv(SZ[g][:, mcs(ck)]))
            for b in range(B):
                op_ps = ppt()
                lhs = w_op[b * 64:(b + 1) * 64, g * 64:(g + 1) * 64]
                nc.tensor.matmul(op_ps[R0:R0 + 64], lhs,
                                 yh[b * 64:(b + 1) * 64],
                                 start=True, stop=True,
                                 tile_position=(b * 64, R0))
                ymt = ym[b][g // 2]
                xnt = xn[b][g // 2]
                if colm:
                    dst = bass.AP(tensor=ymt.tensor,
                                  offset=ymt.offset + 8 * ck,
                                  ap=[[ymt.ap[0][0], 128], [1, 8],
                                      [56, 56]])[R0:R0 + 64]
                    xnsrc = bass.AP(tensor=xnt.tensor,
                                    offset=xnt.offset + 8 * ck,
                                    ap=[[xnt.ap[0][0], 128], [1, 8],
                                        [56, 56]])[R0:R0 + 64]
                else:
                    dst = hv(ymt[R0:R0 + 64, mcs(ck)])
                    xnsrc = hv(xnt[R0:R0 + 64, mcs(ck)])
                nc.vector.scalar_tensor_tensor(
                    dst, op_ps[R0:R0 + 64], gate[b][g // 2][R0:R0 + 64],
                    xnsrc, OP.mult, OP.mult)

    # LNym stats split into two passes: the t-column finished by groups 0/1
    # is reduced early (its matmuls fill the scan(2)/scan(3) PE stalls); the
    # t=1 pass accumulates into the same arena after group 3
    ymar_h = {}

    def ym_stats_pass(tcol, first):
        if first:
            # allocated lazily: must enter the st_ar ring AFTER arn(0)
            ymar_h["t"] = big.tile([2, 2, L], BF16, tag="st_ar", name="ymar")
        ymar = ymar_h["t"]
        for ck in range(NCK):
            mps = ps2.tile([2, CK], F32, tag="pp2")
            sps = ps2.tile([2, CK], F32, tag="pp2")
            for i in range(B):
                tl = ym[i][tcol]
                lw = w_stF_h[:, 2 * i:2 * i + 2]
                sq = sc.tile([128, CK], BF16, tag="cs1", bufs=3)
                nc.scalar.activation(sq, tl[:, cs(ck)], AF.Square)
                nc.tensor.matmul(mps, lw, tl[:, cs(ck)],
                                 start=(i == 0), stop=(i == B - 1))
                nc.tensor.matmul(sps, lw, sq,
                                 start=(i == 0), stop=(i == B - 1))
            if first:
                nc.vector.tensor_copy(ymar[:, 0, cs(ck)], mps)
                nc.scalar.copy(ymar[:, 1, cs(ck)], sps)
            else:
                nc.vector.tensor_add(ymar[:, 0, cs(ck)],
                                     ymar[:, 0, cs(ck)], mps)
                nc.vector.tensor_add(ymar[:, 1, cs(ck)],
                                     ymar[:, 1, cs(ck)], sps)

    # software-pipelined group schedule: conv work of group g+1 is emitted
    # before the scan of group g so PE (and the DVE FIFO) have dense work
    # while the 6.7us scan runs
    conv_stage(0)
    for g in range(G):
        comp_stage(g)
        if g + 1 < G:
            conv_stage(g + 1)
        scan_stage(g)
        post_stage(g)
        if g == 1:
            ym_stats_pass(0, True)

    # ======== LN1(ym) in-place -> ymhat; proj; x2 = xt + proj + b ========
    # reload x in slot-death order (bigf1 frees at scan(2), bigf2 at post(2),
    # bigf3 at scan(3), bigf0 at post(3)) so the sync DMA FIFO never
    # head-of-line blocks on the last-freed slot
    for b, t in ((0, 1), (1, 0), (1, 1), (0, 0)):
        xt[b][t] = big.tile([128, L], F32, tag=f"bigf{b * T + t}",
                            name=f"xt2_{b * T + t}")
        nc.sync.dma_start(xt[b][t], ins["xt"][b, t])
    ym_stats_pass(1, False)
    rym, mym = stats_finish(ymar_h["t"])
    for b in range(B):
        lw = w_bci[:, b * 128:(b + 1) * 128]
        ln_apply(rym, mym, [(ym[b][t], ym[b][t]) for t in range(T)], lw)
    for b in range(B):
        for t in range(T):
            for ck in range(NM):
                # psi ring (idle here): keeps proj off the pp ring so it
                # pipelines per-chunk with the LNym apply instead of
                # queueing behind all of its slot allocations
                xp = ppt(pool=psi)
                for kt in range(T):
                    lhs = w_pj[:, (t * T + kt) * 128:(t * T + kt + 1) * 128]
                    mmsplit(xp, lhs, ym[b][kt][:, mcs(ck)],
                            start=(kt == 0), stop=(kt == T - 1))
                nc.vector.scalar_tensor_tensor(
                    hv(xt[b][t][:, mcs(ck)]), xp, V(f"pjb{t}"),
                    hv(xt[b][t][:, mcs(ck)]), OP.add, OP.add)

    # ======== LN2 -> xhat2 (xn slots); spill x2 to DRAM ========
    r2, m2 = ln_stats([(xt[b][t], b) for b in range(B) for t in range(T)], True)
    xh2 = [[big.tile([128, L], BF16, tag=f"xn{b * T + t}",
                     name=f"xh2_{b * T + t}") for t in range(T)]
           for b in range(B)]
    for b in range(B):
        lw = w_bci[:, b * 128:(b + 1) * 128]
        ln_apply(r2, m2, [(xt[b][t], xh2[b][t]) for t in range(T)], lw)

    # ======== MLP ======== (x2 stays resident in the bigf slots; gel lives
    # in the dead ym/U/SZ slots, so no DRAM spill round-trip is needed)
    for b in range(B):
        gels = ([big.tile([128, L], BF16, tag=f"bigG{j}", name=f"gel{b}_{j}")
                 for j in range(4)]
                + [grp.tile([128, L], BF16, tag="ub", bufs=2,
                            name=f"gelu{b}_{j}") for j in range(2)]
                + [grp.tile([128, L], BF16, tag="szb", bufs=2,
                            name=f"gelz{b}_{j}") for j in range(2)])
        for s in range(HS):
            f1s = grp.tile([128, 2 * 128], BF16, tag=f"f1s{s % 2}",
                           name=f"f1s{s % 2}", bufs=1)
            nc.gpsimd.dma_start(
                f1s[:, 0:128], ins["w_f1"][:, (0 * HS + s) * 128:(0 * HS + s + 1) * 128])
            nc.gpsimd.dma_start(
                f1s[:, 128:256], ins["w_f1"][:, (1 * HS + s) * 128:(1 * HS + s + 1) * 128])
            cvhd = grp.tile([128, 9 * 128], BF16, tag=f"cvd{s % 2}",
                            name=f"cvhd{s % 2}", bufs=1)
            nc.gpsimd.dma_start(cvhd,
                                ins["w_cvh"][:, s * 9 * 128:(s + 1) * 9 * 128])
            padt = padz[s % 2]
            for ck in range(NM):
                hp = ppt()
                for kt in range(T):
                    lhs = f1s[:, kt * 128:(kt + 1) * 128]
                    mmsplit(hp, lhs, xh2[b][kt][:, mcs(ck)],
                            start=(kt == 0), stop=(kt == T - 1))
                dst = bass.AP(tensor=padt.tensor,
                              offset=padt.offset + (1 + 8 * ck) * PW + 1,
                              ap=[padt.ap[0], [PW, 8], [1, 56]])
                nc.vector.tensor_copy(dst, hp)
            for ck in range(NM):
                cvp = ppt(pool=psf if ck % 2 == 0 else psi)
                for k in range(9):
                    dy, dx = k // 3, k % 3
                    lhs = cvhd[:, k * 128:(k + 1) * 128]
                    rhs_ = bass.AP(
                        tensor=padt.tensor,
                        offset=padt.offset + (8 * ck + dy) * PW + dx,
                        ap=[padt.ap[0], [PW, 8], [1, 56]])
                    nc.tensor.matmul(cvp, lhs, rhs_,
                                     start=(k == 0), stop=(k == 8))
                nc.scalar.activation(hv(gels[s][:, mcs(ck)]), cvp,
                                     AF.Gelu, bias=V(f"dwb{s}"))
        for t in range(T):
            for ck in range(NM):
                xp = ppt()
                for s in range(HS):
                    lhs = w_f2[:, (s * T + t) * 128:(s * T + t + 1) * 128]
                    mmsplit(xp, lhs, gels[s][:, mcs(ck)],
                            start=(s == 0), stop=(s == HS - 1))
                x3 = sc.tile([128, MCK], F32, tag="c1", bufs=4)
                nc.vector.scalar_tensor_tensor(
                    hv(x3), xp, V(f"f2b{t}"), hv(xt[b][t][:, mcs(ck)]),
                    OP.add, OP.add)
                nc.sync.dma_start(outs["out"][b, t, :, mcs(ck)], x3)


# =================================================================
# Runner: full-input kernel() entry point. Shards batch over 8 cores,
# compiles the Bass module once, runs SPMD via PJRT/axon, gathers.
# =================================================================
import concourse.bacc as bacc
import concourse.bass_utils as bass_utils

N_CORES = 8
_CACHE = {}


def _build_nc():
    if "nc" in _CACHE:
        return _CACHE["nc"]
    nc = bacc.Bacc("TRN2", debug=False, num_devices=N_CORES)
    ispec = input_specs()
    ins = {}
    for name, (shape, dt) in ispec.items():
        mdt = F32 if dt == np.float32 else BF16
        ins[name] = nc.dram_tensor(name, shape, mdt, kind="ExternalInput").ap()
    out = nc.dram_tensor("out", (B, T, 128, L), F32, kind="ExternalOutput").ap()
    import concourse.tile as _tile
    with _tile.TileContext(nc) as tc:
        with ExitStack() as ctx:
            body(ctx, tc, {"out": out}, ins)
    nc.compile()
    _CACHE["nc"] = nc
    return nc


def make_in_maps(inputs):
    x = np.asarray(inputs["x"], dtype=np.float32)
    return [host_prep(x[c * B:(c + 1) * B], inputs) for c in range(N_CORES)]


def kernel(**inputs):
    nc = _build_nc()
    in_maps = make_in_maps(inputs)
    res = bass_utils.run_bass_kernel_spmd(nc, in_maps,
                                          core_ids=list(range(N_CORES)))
    outs = []
    for c in range(N_CORES):
        o = res.results[c]["out"].reshape(B, C, L).transpose(0, 2, 1)
        outs.append(o)
    return np.ascontiguousarray(np.concatenate(outs, axis=0))

